# revision 13
# baseline (speedup 1.0000x reference)
"""DLSMN scatter-memory + cache self-attention kernel for Trainium2.

Data-parallel over batch: batch b runs on NeuronCore b (8 cores), no
collectives.  Inside one core (one batch):

  phase A: per 128-token tile of y: PE-transpose y -> yT chunks, fused
           matmuls  [W_write | (W_slot,W_gate)]  (fp32r), gumbel-softmax
           routing via exp(logits*gamma - ln(-ln(u+eps)+eps)) (single
           Ln/Exp ACT table set), weighted-scatter matmul with a leading
           ones column in the rhs so the write-mass comes out of the same
           accumulation for free.
  phase B: slot update  upd = (1-g)*DECAY*old + g*updates/(mass+eps).
  phase C: PE-transpose cache2 -> cache2T (bf16).
  phase D: q/k/v projections in bf16 (qT,kT transposed layout; v natural).
  phase E: attention computed transposed: attT[m,n] tiles; softmax has no
           max-subtraction (logits are provably tiny); denominators via
           col-tiled ones-matmuls (4 heads concurrent in the PE array);
           ao^T accumulated in PSUM; normalization by exp(-ln(den)).
  phase F: output projection + residual + layernorm (fused DVE
           scalar_tensor_tensor with accum_out row sums).
"""

import numpy as np

import concourse.bacc as bacc
import concourse.mybir as mybir
import concourse.tile as tile
from concourse.bass_utils import run_bass_kernel_spmd
from concourse.masks import make_identity

F32 = mybir.dt.float32
F32R = mybir.dt.float32r
BF16 = mybir.dt.bfloat16
AF = mybir.ActivationFunctionType
ALU = mybir.AluOpType

B = 8
S = 2048
D = 1024
DC = 512
K = 256
L = 8
H = 4
HD = 128
N = L * K
LAYER_IDX = 3
DECAY = 0.9
EPS = 1e-6
ST = S // 128  # 16 token tiles
NT = N // 128  # 16 slot tiles
DCH = D // 128  # 8 d_model chunks
CL = 256  # attention n-chunk length
NCH = N // CL  # 8 attention chunks
ATT_SCALE = float(1.0 / np.sqrt(np.float32(HD)))

_INPUT_SPECS = {
    "y": (S, D), "cache": (N, DC), "gumbel_u": (S, K),
    "W_gate": (D, 1), "b_gate": (1,), "W_slot": (D, K), "b_slot": (K,),
    "gamma": (1,), "W_write": (D, DC), "b_write": (DC,),
    "Wq": (DC, DC), "bq": (DC,), "Wk": (DC, DC), "bk": (DC,),
    "Wv": (DC, DC), "bv": (DC,), "Wo": (DC, DC), "bo": (DC,),
    "ln_g": (DC,), "ln_b": (DC,),
}


def _r(ap):
    return ap.bitcast(F32R)


def _build():
    nc = bacc.Bacc("TRN2", target_bir_lowering=False, debug=False, num_devices=B)

    a = {
        name: nc.dram_tensor(name, list(shape), F32, kind="ExternalInput").ap()
        for name, shape in _INPUT_SPECS.items()
    }
    out_dram = nc.dram_tensor("out", [N, DC], F32, kind="ExternalOutput").ap()

    y3 = a["y"].rearrange("(t p) d -> p t d", p=128)
    gum3 = a["gumbel_u"].rearrange("(t p) k -> p t k", p=128)
    cache3 = a["cache"].rearrange("(t p) d -> p t d", p=128)
    out3 = out_dram.rearrange("(t p) d -> p t d", p=128)

    with tile.TileContext(nc) as tc:
        with (
            tc.tile_pool(name="const", bufs=1) as const,
            tc.tile_pool(name="cachep", bufs=1) as cachep,
        ):
            ident = const.tile([128, 128], F32)
            make_identity(nc, ident)
            ones_row_f = const.tile([1, DC], F32)
            nc.vector.memset(ones_row_f, 1.0)
            ones_col2_f = const.tile([128, 2], F32)
            nc.vector.memset(ones_col2_f, 1.0)
            ones_row = const.tile([1, DC], F32R)
            nc.vector.tensor_copy(out=ones_row, in_=ones_row_f)
            ones_row_bf = const.tile([1, DC], BF16)
            nc.vector.memset(ones_row_bf, 1.0)
            ones_col_bf = const.tile([128, 1], BF16)
            nc.vector.memset(ones_col_bf, 1.0)
            eps8_t = const.tile([128, 1], F32)
            nc.vector.memset(eps8_t, 1e-8)
            eps5_t = const.tile([128, 1], F32)
            nc.vector.memset(eps5_t, 1e-5)
            gamma_t = const.tile([128, 1], F32)
            nc.sync.dma_start(out=gamma_t, in_=a["gamma"].unsqueeze(0).to_broadcast([128, 1]))
            lng_bc = const.tile([128, DC], F32)
            nc.sync.dma_start(out=lng_bc, in_=a["ln_g"].unsqueeze(0).to_broadcast([128, DC]))
            lnb_bc = const.tile([128, DC], F32)
            nc.sync.dma_start(out=lnb_bc, in_=a["ln_b"].unsqueeze(0).to_broadcast([128, DC]))
            bwr_row = const.tile([1, DC], F32R)
            nc.gpsimd.dma_start(out=bwr_row, in_=a["b_write"].unsqueeze(0))
            bsg_row = const.tile([1, K + 2], F32R)
            nc.gpsimd.dma_start(out=bsg_row[:, 0:K], in_=a["b_slot"].unsqueeze(0))
            nc.gpsimd.dma_start(out=bsg_row[:, K:K + 1], in_=a["b_gate"].unsqueeze(0))
            nc.gpsimd.dma_start(out=bsg_row[:, K + 1:K + 2], in_=a["b_gate"].unsqueeze(0))
            # bf16 bias rows for the attention-side projections
            bqr = const.tile([1, DC], BF16)
            nc.gpsimd.dma_start(out=bqr, in_=a["bq"].unsqueeze(0))
            bkr = const.tile([1, DC], BF16)
            nc.gpsimd.dma_start(out=bkr, in_=a["bk"].unsqueeze(0))
            bvr = const.tile([1, DC], BF16)
            nc.gpsimd.dma_start(out=bvr, in_=a["bv"].unsqueeze(0))
            bor = const.tile([1, DC], BF16)
            nc.gpsimd.dma_start(out=bor, in_=a["bo"].unsqueeze(0))

            cache_sb = cachep.tile([128, NT, DC], F32)
            nc.sync.dma_start(out=cache_sb, in_=cache3)

            # ---------------- phase A + B: selection & scatter write ------
            with (
                tc.tile_pool(name="wA", bufs=1) as wA,
                tc.tile_pool(name="pA", bufs=2) as pA,
                tc.tile_pool(name="pAs", bufs=3) as pAs,
                tc.tile_pool(name="psU", bufs=1, space="PSUM") as psU,
                tc.tile_pool(name="psA", bufs=1, space="PSUM") as psA,
                tc.tile_pool(name="psT", bufs=2, space="PSUM") as psT,
            ):
                wwr = wA.tile([128, DCH, DC], F32R)
                nc.gpsimd.dma_start(out=wwr, in_=a["W_write"].rearrange("(c p) d -> p c d", p=128))
                wsg = wA.tile([128, DCH, K + 2], F32R)
                nc.gpsimd.dma_start(out=wsg[:, :, 0:K], in_=a["W_slot"].rearrange("(c p) k -> p c k", p=128))
                nc.gpsimd.dma_start(out=wsg[:, :, K:K + 1], in_=a["W_gate"].rearrange("(c p) o -> p c o", p=128))
                nc.gpsimd.dma_start(out=wsg[:, :, K + 1:K + 2], in_=a["W_gate"].rearrange("(c p) o -> p c o", p=128))

                # persistent scatter accumulators: [ones|wv] x w  ->  [mass | updates]
                ps_ua = [psU.tile([128, K + 2], F32, name=f"ua{kc}", tag=f"ua{kc}")
                         for kc in range(2)]
                ps_ub = [psU.tile([128, K], F32, name=f"ub{kc}", tag=f"ub{kc}")
                         for kc in range(2)]

                for i in range(ST):
                    y_t = pA.tile([128, D], F32, tag="y")
                    nc.sync.dma_start(out=y_t, in_=y3[:, i, :])
                    gum = pA.tile([128, K], F32, tag="gum")
                    nc.sync.dma_start(out=gum, in_=gum3[:, i, :])

                    # transpose y tile -> yT (8 chunks of [128d, 128s])
                    yT = pA.tile([128, D], F32R, tag="yT")
                    for g in range(2):
                        tr = psT.tile([128, 512], F32, tag="tr")
                        for cc in range(4):
                            c = 4 * g + cc
                            nc.tensor.transpose(
                                tr[:, cc * 128:(cc + 1) * 128],
                                y_t[:, c * 128:(c + 1) * 128],
                                ident,
                            )
                        nc.any.tensor_copy(out=yT[:, g * 512:(g + 1) * 512], in_=tr)

                    # fused write_vals / (logits, gate) matmuls
                    ps_wv = psA.tile([128, DC], F32, tag="wv")
                    for c in range(DCH):
                        nc.tensor.matmul(
                            ps_wv, yT[:, c * 128:(c + 1) * 128], wwr[:, c, :],
                            start=(c == 0), stop=False,
                        )
                    nc.tensor.matmul(ps_wv, ones_row[:, 0:128], bwr_row,
                                     start=False, stop=True)
                    ps_lg = psA.tile([128, K + 2], F32, tag="lg")
                    for c in range(DCH):
                        nc.tensor.matmul(
                            ps_lg, yT[:, c * 128:(c + 1) * 128], wsg[:, c, :],
                            start=(c == 0), stop=False,
                        )
                    nc.tensor.matmul(ps_lg, ones_row[:, 0:128], bsg_row,
                                     start=False, stop=True)

                    # gumbel: lnz = ln(-ln(u+1e-8)+1e-8);  t = gamma*logits - lnz
                    lnu = pAs.tile([128, K], F32, tag="lnu")
                    nc.scalar.activation(lnu, gum, AF.Ln, bias=eps8_t)
                    lnz = pAs.tile([128, K], F32, tag="lnz")
                    nc.scalar.activation(lnz, lnu, AF.Ln, bias=eps8_t, scale=-1.0)
                    t_sb = pAs.tile([128, K], F32, tag="tsb")
                    nc.vector.scalar_tensor_tensor(
                        out=t_sb, in0=ps_lg[:, 0:K], scalar=gamma_t, in1=lnz,
                        op0=ALU.mult, op1=ALU.subtract,
                    )

                    # scores = sigmoid(gate) = 1/(1+exp(-gate))
                    sc_e = pAs.tile([128, 1], F32, tag="sce")
                    nc.scalar.activation(sc_e, ps_lg[:, K:K + 1], AF.Exp, scale=-1.0)
                    sc1 = pAs.tile([128, 1], F32, tag="sc1")
                    nc.vector.tensor_scalar_add(sc1, sc_e, 1.0)
                    scores = pAs.tile([128, 1], F32, tag="scores")
                    nc.vector.reciprocal(scores, sc1)

                    # p_unnorm = exp(t), row-sum fused; w = p_unnorm*(scores/rowsum)
                    p_un = pAs.tile([128, K], F32, tag="pun")
                    rs = pAs.tile([128, 1], F32, tag="rs")
                    nc.scalar.activation(p_un, t_sb, AF.Exp, accum_out=rs)
                    rrs = pAs.tile([128, 1], F32, tag="rrs")
                    nc.vector.reciprocal(rrs, rs)
                    s2 = pAs.tile([128, 1], F32, tag="s2")
                    nc.vector.tensor_tensor(s2, scores, rrs, ALU.mult)
                    w_sb = pAs.tile([128, K], F32R, tag="wsb")
                    nc.vector.tensor_scalar_mul(w_sb, p_un, s2)

                    # wv_sb = [ones | write_vals]
                    wv_sb = pAs.tile([128, DC + 2], F32R, tag="wvsb")
                    nc.vector.tensor_copy(out=wv_sb[:, 0:2], in_=ones_col2_f)
                    nc.any.tensor_copy(out=wv_sb[:, 2:DC + 2], in_=ps_wv)

                    for kc in range(2):
                        lhs = w_sb[:, kc * 128:(kc + 1) * 128]
                        nc.tensor.matmul(ps_ua[kc], lhs, wv_sb[:, 0:K + 2],
                                         start=(i == 0), stop=(i == ST - 1))
                        nc.tensor.matmul(ps_ub[kc], lhs, wv_sb[:, K + 2:DC + 2],
                                         start=(i == 0), stop=(i == ST - 1))

                # ------- phase B: slot update, overwrite cache rows -------
                base_t = LAYER_IDX * K // 128  # n-tile 6
                for kc in range(2):
                    mass = pAs.tile([128, 1], F32, tag="mass")
                    nc.any.tensor_copy(out=mass, in_=ps_ua[kc][:, 0:1])
                    m1 = pAs.tile([128, 1], F32, tag="m1")
                    nc.vector.tensor_scalar_add(m1, mass, EPS)
                    rm = pAs.tile([128, 1], F32, tag="rm")
                    nc.vector.reciprocal(rm, m1)
                    m2 = pAs.tile([128, 1], F32, tag="m2")
                    nc.vector.tensor_scalar_add(m2, mass, 1.0)
                    rg = pAs.tile([128, 1], F32, tag="rg")
                    nc.vector.reciprocal(rg, m2)
                    g_t = pAs.tile([128, 1], F32, tag="gt")
                    nc.vector.tensor_tensor(g_t, mass, rg, ALU.mult)
                    co = pAs.tile([128, 1], F32, tag="co")
                    nc.vector.tensor_scalar(co, g_t, -DECAY, DECAY, ALU.mult, ALU.add)
                    cn = pAs.tile([128, 1], F32, tag="cn")
                    nc.vector.tensor_tensor(cn, g_t, rm, ALU.mult)

                    told = pAs.tile([128, DC], F32, tag="told")
                    nc.vector.tensor_scalar_mul(told, cache_sb[:, base_t + kc, :], co)
                    nc.vector.scalar_tensor_tensor(
                        out=cache_sb[:, base_t + kc, 0:K],
                        in0=ps_ua[kc][:, 2:K + 2], scalar=cn, in1=told[:, 0:K],
                        op0=ALU.mult, op1=ALU.add,
                    )
                    nc.vector.scalar_tensor_tensor(
                        out=cache_sb[:, base_t + kc, K:DC],
                        in0=ps_ub[kc], scalar=cn, in1=told[:, K:DC],
                        op0=ALU.mult, op1=ALU.add,
                    )

            # ---------------- phases C-F ----------------------------------
            with (
                tc.tile_pool(name="woP", bufs=1) as woP,
                tc.tile_pool(name="aoP", bufs=1) as aoP,
            ):
                wo_sb = woP.tile([128, H, DC], BF16)
                nc.gpsimd.dma_start(out=wo_sb, in_=a["Wo"].rearrange("(c p) d -> p c d", p=128))
                aoT = aoP.tile([128, H, N], BF16)

                with (
                    tc.tile_pool(name="c2tP", bufs=1) as c2tP,
                    tc.tile_pool(name="wqkvP", bufs=1) as wqkvP,
                    tc.tile_pool(name="qkvP", bufs=1) as qkvP,
                ):
                    # ------- phase C: cache2 -> cache2T (bf16) -----------
                    c2t = c2tP.tile([128, 4, N], BF16)
                    with tc.tile_pool(name="psC", bufs=2, space="PSUM") as psC:
                        for j in range(4):
                            for tg in range(4):
                                ps = psC.tile([128, 512], F32, tag="ctr")
                                for tt in range(4):
                                    t = tg * 4 + tt
                                    nc.tensor.transpose(
                                        ps[:, tt * 128:(tt + 1) * 128],
                                        cache_sb[:, t, j * 128:(j + 1) * 128],
                                        ident,
                                    )
                                nc.any.tensor_copy(
                                    out=c2t[:, j, tg * 512:(tg + 1) * 512], in_=ps)

                    # ------- phase D: q/k/v projections (bf16) -----------
                    wq_sb = wqkvP.tile([128, 4, DC], BF16)
                    nc.gpsimd.dma_start(out=wq_sb, in_=a["Wq"].rearrange("(c p) d -> p c d", p=128))
                    wk_sb = wqkvP.tile([128, 4, DC], BF16)
                    nc.gpsimd.dma_start(out=wk_sb, in_=a["Wk"].rearrange("(c p) d -> p c d", p=128))
                    wv_w = wqkvP.tile([128, 4, DC], BF16)
                    nc.gpsimd.dma_start(out=wv_w, in_=a["Wv"].rearrange("(c p) d -> p c d", p=128))

                    qT = qkvP.tile([128, H, N], BF16)
                    kT = qkvP.tile([128, H, N], BF16)
                    v_sb = qkvP.tile([128, NT, DC], BF16)

                    with tc.tile_pool(name="psD", bufs=3, space="PSUM") as psD:
                        for dst, w_t, b_t in ((qT, wq_sb, bqr), (kT, wk_sb, bkr)):
                            for h in range(H):
                                for c in range(4):
                                    ps = psD.tile([128, 512], F32, tag="qk")
                                    for j in range(4):
                                        nc.tensor.matmul(
                                            ps, w_t[:, j, h * 128:(h + 1) * 128],
                                            c2t[:, j, c * 512:(c + 1) * 512],
                                            start=(j == 0), stop=False,
                                        )
                                    nc.tensor.matmul(
                                        ps, b_t[:, h * 128:(h + 1) * 128],
                                        ones_row_bf[:, 0:512], start=False, stop=True,
                                    )
                                    nc.any.tensor_copy(
                                        out=dst[:, h, c * 512:(c + 1) * 512], in_=ps)
                        for m in range(NT):
                            ps = psD.tile([128, DC], F32, tag="v")
                            for j in range(4):
                                nc.tensor.matmul(
                                    ps, c2t[:, j, m * 128:(m + 1) * 128], wv_w[:, j, :],
                                    start=(j == 0), stop=False,
                                )
                            nc.tensor.matmul(ps, ones_row_bf[:, 0:128], bvr,
                                             start=False, stop=True)
                            nc.any.tensor_copy(out=v_sb[:, m, :], in_=ps)

                    # ------- phase E: attention (transposed layout) ------
                    with (
                        tc.tile_pool(name="pE", bufs=3) as pE,
                        tc.tile_pool(name="pEs", bufs=2) as pEs,
                        tc.tile_pool(name="psAtt", bufs=1, space="PSUM") as psAtt,
                        tc.tile_pool(name="psAo", bufs=1, space="PSUM") as psAo,
                        tc.tile_pool(name="psDen", bufs=2, space="PSUM") as psDen,
                        tc.tile_pool(name="psBc", bufs=2, space="PSUM") as psBc,
                    ):
                        ps_aoA = psAo.tile([128, 512], F32, tag="aoA")
                        ps_aoB = psAo.tile([128, 512], F32, tag="aoB")
                        for c in range(NCH):
                            ps_den = psDen.tile([128, CL], F32, tag="den")
                            for m in range(NT):
                                ps_a = psAtt.tile([128, H * CL], F32, tag="att")
                                for h in range(H):
                                    nc.tensor.matmul(
                                        ps_a[:, h * CL:(h + 1) * CL],
                                        kT[:, h, m * 128:(m + 1) * 128],
                                        qT[:, h, c * CL:(c + 1) * CL],
                                        start=True, stop=True,
                                    )
                                pT = pE.tile([128, H * CL], BF16, tag="pT")
                                nc.scalar.activation(pT, ps_a, AF.Exp, scale=ATT_SCALE)
                                for h in range(H):
                                    ps_ao = ps_aoA if h < 2 else ps_aoB
                                    # one accumulation group per PSUM bank: the
                                    # bank-wide zero region from the h-even start
                                    # makes the h-odd m==0 write an overwrite.
                                    nc.tensor.matmul(
                                        ps_ao[:, (h % 2) * CL:(h % 2 + 1) * CL],
                                        v_sb[:, m, h * 128:(h + 1) * 128],
                                        pT[:, h * CL:(h + 1) * CL],
                                        start=(m == 0 and h % 2 == 0),
                                        stop=(m == NT - 1 and h % 2 == 1),
                                    )
                                for h in range(H):
                                    nc.tensor.matmul(
                                        ps_den[32 * h:32 * h + 1, :],
                                        ones_col_bf,
                                        pT[:, h * CL:(h + 1) * CL],
                                        start=(m == 0), stop=(m == NT - 1),
                                        tile_position=(0, 32 * h),
                                    )
                            # denominators -> 1/den (via exp(-ln)) -> broadcast
                            den_sb = pEs.tile([128, CL], F32, tag="densb")
                            for h in range(H):
                                nc.any.tensor_copy(
                                    out=den_sb[32 * h:32 * h + 1, :],
                                    in_=ps_den[32 * h:32 * h + 1, :],
                                )
                            den4 = pEs.tile([1, H * CL], F32, tag="den4")
                            for h in range(H):
                                nc.sync.dma_start(
                                    out=den4[:, h * CL:(h + 1) * CL],
                                    in_=den_sb[32 * h:32 * h + 1, :],
                                )
                            lnden = pEs.tile([1, H * CL], F32, tag="lnden")
                            nc.scalar.activation(lnden, den4, AF.Ln)
                            rden = pEs.tile([1, H * CL], F32R, tag="rden")
                            nc.scalar.activation(rden, lnden, AF.Exp, scale=-1.0)
                            for h in range(H):
                                bc = psBc.tile([128, CL], F32, tag="bc")
                                nc.tensor.matmul(
                                    bc,
                                    ones_row[:, 0:128],
                                    rden[:, h * CL:(h + 1) * CL],
                                    start=True, stop=True,
                                )
                                bc_sb = pEs.tile([128, CL], F32, tag="bcsb")
                                nc.any.tensor_copy(out=bc_sb, in_=bc)
                                ps_ao = ps_aoA if h < 2 else ps_aoB
                                nc.vector.scalar_tensor_tensor(
                                    out=aoT[:, h, c * CL:(c + 1) * CL],
                                    in0=ps_ao[:, (h % 2) * CL:(h % 2 + 1) * CL],
                                    scalar=1.0, in1=bc_sb,
                                    op0=ALU.mult, op1=ALU.mult,
                                )

                # ------- phase F: o-proj + residual + layernorm ----------
                with (
                    tc.tile_pool(name="pF", bufs=2) as pF,
                    tc.tile_pool(name="psF", bufs=2, space="PSUM") as psF,
                ):
                    for t in range(NT):
                        ps_o = psF.tile([128, DC], F32, tag="o")
                        for h in range(H):
                            nc.tensor.matmul(
                                ps_o, aoT[:, h, t * 128:(t + 1) * 128], wo_sb[:, h, :],
                                start=(h == 0), stop=False,
                            )
                        nc.tensor.matmul(ps_o, ones_row_bf[:, 0:128], bor,
                                         start=False, stop=True)
                        r = pF.tile([128, DC], F32, tag="r")
                        rsum = pF.tile([128, 1], F32, tag="rsum")
                        nc.vector.scalar_tensor_tensor(
                            out=r, in0=ps_o, scalar=1.0, in1=cache_sb[:, t, :],
                            op0=ALU.mult, op1=ALU.add, accum_out=rsum,
                        )
                        mean = pF.tile([128, 1], F32, tag="mean")
                        nc.vector.tensor_scalar_mul(mean, rsum, 1.0 / DC)
                        scratch = pF.tile([128, DC], F32, tag="scratch")
                        ssq = pF.tile([128, 1], F32, tag="ssq")
                        nc.vector.scalar_tensor_tensor(
                            out=scratch, in0=r, scalar=mean, in1=r,
                            op0=ALU.subtract, op1=ALU.mult, accum_out=ssq,
                        )
                        lnv = pF.tile([128, 1], F32, tag="lnv")
                        nc.scalar.activation(lnv, ssq, AF.Ln, scale=1.0 / DC, bias=eps5_t)
                        rstd = pF.tile([128, 1], F32, tag="rstd")
                        nc.scalar.activation(rstd, lnv, AF.Exp, scale=-0.5)
                        t1 = pF.tile([128, DC], F32, tag="t1")
                        nc.vector.tensor_scalar(t1, r, mean, rstd, ALU.subtract, ALU.mult)
                        t2 = pF.tile([128, DC], F32, tag="t2")
                        nc.vector.scalar_tensor_tensor(
                            out=t2, in0=t1, scalar=1.0, in1=lng_bc,
                            op0=ALU.mult, op1=ALU.mult,
                        )
                        o_sb = pF.tile([128, DC], F32, tag="osb")
                        nc.vector.scalar_tensor_tensor(
                            out=o_sb, in0=t2, scalar=1.0, in1=lnb_bc,
                            op0=ALU.mult, op1=ALU.add,
                        )
                        nc.sync.dma_start(out=out3[:, t, :], in_=o_sb)

    nc.compile()
    return nc


_NC_CACHE = {}


def _get_nc():
    if "nc" not in _NC_CACHE:
        _NC_CACHE["nc"] = _build()
    return _NC_CACHE["nc"]


def _in_maps(inputs):
    per_batch = {"y", "cache", "gumbel_u"}
    maps = []
    for b in range(B):
        m = {}
        for name in _INPUT_SPECS:
            arr = np.ascontiguousarray(np.asarray(inputs[name], dtype=np.float32))
            m[name] = arr[b] if name in per_batch else arr
        maps.append(m)
    return maps


def _execute(inputs, trace=False):
    nc = _get_nc()
    res = run_bass_kernel_spmd(nc, _in_maps(inputs), list(range(B)), trace=trace)
    out = np.stack([res.results[b]["out"] for b in range(B)]).astype(np.float32)
    return out, res


def kernel(**inputs) -> np.ndarray:
    out, _ = _execute(inputs)
    return out


# revision 14
# speedup vs baseline: 1.2838x; 1.2838x over previous
"""DLSMN scatter-memory + cache self-attention kernel for Trainium2.

Data-parallel over batch: batch b runs on NeuronCore b (8 cores), no
collectives.  Inside one core (one batch):

  phase A: per 128-token tile of y: PE-transpose y -> yT chunks, fused
           matmuls  [W_write | (W_slot,W_gate)]  (fp32r), gumbel-softmax
           routing via exp(logits*gamma - ln(-ln(u+eps)+eps)) (single
           Ln/Exp ACT table set), weighted-scatter matmul with a leading
           ones column in the rhs so the write-mass comes out of the same
           accumulation for free.
  phase B: slot update  upd = (1-g)*DECAY*old + g*updates/(mass+eps).
  phase C: PE-transpose cache2 -> cache2T (bf16).
  phase D: q/k/v projections in bf16 (qT,kT transposed layout; v natural).
  phase E: attention computed transposed: attT[m,n] tiles; softmax has no
           max-subtraction (logits are provably tiny); denominators via
           col-tiled ones-matmuls (4 heads concurrent in the PE array);
           ao^T accumulated in PSUM; normalization by exp(-ln(den)).
  phase F: output projection + residual + layernorm (fused DVE
           scalar_tensor_tensor with accum_out row sums).
"""

import numpy as np

import concourse.bacc as bacc
import concourse.mybir as mybir
import concourse.tile as tile
from concourse.bass_utils import run_bass_kernel_spmd
from concourse.masks import make_identity

F32 = mybir.dt.float32
F32R = mybir.dt.float32r
BF16 = mybir.dt.bfloat16
AF = mybir.ActivationFunctionType
ALU = mybir.AluOpType

B = 8
S = 2048
D = 1024
DC = 512
K = 256
L = 8
H = 4
HD = 128
N = L * K
LAYER_IDX = 3
DECAY = 0.9
EPS = 1e-6
ST = S // 128  # 16 token tiles
NT = N // 128  # 16 slot tiles
DCH = D // 128  # 8 d_model chunks
CL = 256  # attention n-chunk length
NCH = N // CL  # 8 attention chunks
ATT_SCALE = float(1.0 / np.sqrt(np.float32(HD)))

_INPUT_SPECS = {
    "y": (S, D), "cache": (N, DC), "gumbel_u": (S, K),
    "W_gate": (D, 1), "b_gate": (1,), "W_slot": (D, K), "b_slot": (K,),
    "gamma": (1,), "W_write": (D, DC), "b_write": (DC,),
    "Wq": (DC, DC), "bq": (DC,), "Wk": (DC, DC), "bk": (DC,),
    "Wv": (DC, DC), "bv": (DC,), "Wo": (DC, DC), "bo": (DC,),
    "ln_g": (DC,), "ln_b": (DC,),
}


def _r(ap):
    return ap.bitcast(F32R)


def _build():
    nc = bacc.Bacc("TRN2", target_bir_lowering=False, debug=False, num_devices=B)

    a = {
        name: nc.dram_tensor(name, list(shape), F32, kind="ExternalInput").ap()
        for name, shape in _INPUT_SPECS.items()
    }
    out_dram = nc.dram_tensor("out", [N, DC], F32, kind="ExternalOutput").ap()

    y3 = a["y"].rearrange("(t p) d -> p t d", p=128)
    gum3 = a["gumbel_u"].rearrange("(t p) k -> p t k", p=128)
    cache3 = a["cache"].rearrange("(t p) d -> p t d", p=128)
    out3 = out_dram.rearrange("(t p) d -> p t d", p=128)

    with tile.TileContext(nc) as tc:
        with (
            tc.tile_pool(name="const", bufs=1) as const,
            tc.tile_pool(name="cachep", bufs=1) as cachep,
        ):
            ident = const.tile([128, 128], F32)
            make_identity(nc, ident)
            ones_row_f = const.tile([1, DC], F32)
            nc.vector.memset(ones_row_f, 1.0)
            ones_col2_f = const.tile([128, 2], F32)
            nc.vector.memset(ones_col2_f, 1.0)
            ones_row = const.tile([1, DC], F32R)
            nc.vector.tensor_copy(out=ones_row, in_=ones_row_f)
            ones_row_bf = const.tile([1, DC], BF16)
            nc.vector.memset(ones_row_bf, 1.0)
            ones_col_bf = const.tile([128, 1], BF16)
            nc.vector.memset(ones_col_bf, 1.0)
            eps8_t = const.tile([128, 1], F32)
            nc.vector.memset(eps8_t, 1e-8)
            eps5_t = const.tile([128, 1], F32)
            nc.vector.memset(eps5_t, 1e-5)
            gamma_t = const.tile([128, 1], F32)
            nc.sync.dma_start(out=gamma_t, in_=a["gamma"].unsqueeze(0).to_broadcast([128, 1]))
            lng_bc = const.tile([128, DC], F32)
            nc.sync.dma_start(out=lng_bc, in_=a["ln_g"].unsqueeze(0).to_broadcast([128, DC]))
            lnb_bc = const.tile([128, DC], F32)
            nc.sync.dma_start(out=lnb_bc, in_=a["ln_b"].unsqueeze(0).to_broadcast([128, DC]))
            bwr_row = const.tile([1, DC], F32R)
            nc.gpsimd.dma_start(out=bwr_row, in_=a["b_write"].unsqueeze(0))
            bsg_row = const.tile([1, K + 2], F32R)
            nc.gpsimd.dma_start(out=bsg_row[:, 0:K], in_=a["b_slot"].unsqueeze(0))
            nc.gpsimd.dma_start(out=bsg_row[:, K:K + 1], in_=a["b_gate"].unsqueeze(0))
            nc.gpsimd.dma_start(out=bsg_row[:, K + 1:K + 2], in_=a["b_gate"].unsqueeze(0))
            # bf16 bias rows for the attention-side projections
            bqr = const.tile([1, DC], BF16)
            nc.gpsimd.dma_start(out=bqr, in_=a["bq"].unsqueeze(0))
            bkr = const.tile([1, DC], BF16)
            nc.gpsimd.dma_start(out=bkr, in_=a["bk"].unsqueeze(0))
            bvr = const.tile([1, DC], BF16)
            nc.gpsimd.dma_start(out=bvr, in_=a["bv"].unsqueeze(0))
            bor = const.tile([1, DC], BF16)
            nc.gpsimd.dma_start(out=bor, in_=a["bo"].unsqueeze(0))

            cache_sb = cachep.tile([128, NT, DC], F32)
            nc.sync.dma_start(out=cache_sb, in_=cache3)

            # ---------------- phase A + B: selection & scatter write ------
            with (
                tc.tile_pool(name="wA", bufs=1) as wA,
                tc.tile_pool(name="pA", bufs=2) as pA,
                tc.tile_pool(name="pAs", bufs=3) as pAs,
                tc.tile_pool(name="psU", bufs=1, space="PSUM") as psU,
                tc.tile_pool(name="psA", bufs=1, space="PSUM") as psA,
                tc.tile_pool(name="psT", bufs=2, space="PSUM") as psT,
            ):
                wwr = wA.tile([128, DCH, DC], F32R)
                nc.gpsimd.dma_start(out=wwr, in_=a["W_write"].rearrange("(c p) d -> p c d", p=128))
                wsg = wA.tile([128, DCH, K + 2], F32R)
                nc.gpsimd.dma_start(out=wsg[:, :, 0:K], in_=a["W_slot"].rearrange("(c p) k -> p c k", p=128))
                nc.gpsimd.dma_start(out=wsg[:, :, K:K + 1], in_=a["W_gate"].rearrange("(c p) o -> p c o", p=128))
                nc.gpsimd.dma_start(out=wsg[:, :, K + 1:K + 2], in_=a["W_gate"].rearrange("(c p) o -> p c o", p=128))

                # persistent scatter accumulators: [ones|wv] x w  ->  [mass | updates]
                ps_ua = [psU.tile([128, K + 2], F32, name=f"ua{kc}", tag=f"ua{kc}")
                         for kc in range(2)]
                ps_ub = [psU.tile([128, K], F32, name=f"ub{kc}", tag=f"ub{kc}")
                         for kc in range(2)]

                for i in range(ST):
                    y_t = pA.tile([128, D], F32, tag="y")
                    nc.sync.dma_start(out=y_t, in_=y3[:, i, :])
                    gum = pA.tile([128, K], F32, tag="gum")
                    nc.sync.dma_start(out=gum, in_=gum3[:, i, :])

                    # transpose y tile -> yT (8 chunks of [128d, 128s])
                    yT = pA.tile([128, D], F32R, tag="yT")
                    for g in range(2):
                        tr = psT.tile([128, 512], F32, tag="tr")
                        for cc in range(4):
                            c = 4 * g + cc
                            nc.tensor.transpose(
                                tr[:, cc * 128:(cc + 1) * 128],
                                y_t[:, c * 128:(c + 1) * 128],
                                ident,
                            )
                        nc.vector.tensor_copy(out=yT[:, g * 512:(g + 1) * 512], in_=tr)

                    # fused write_vals / (logits, gate) matmuls
                    ps_wv = psA.tile([128, DC], F32, tag="wv")
                    for c in range(DCH):
                        nc.tensor.matmul(
                            ps_wv, yT[:, c * 128:(c + 1) * 128], wwr[:, c, :],
                            start=(c == 0), stop=False,
                        )
                    nc.tensor.matmul(ps_wv, ones_row[:, 0:128], bwr_row,
                                     start=False, stop=True)
                    ps_lg = psA.tile([128, K + 2], F32, tag="lg")
                    for c in range(DCH):
                        nc.tensor.matmul(
                            ps_lg, yT[:, c * 128:(c + 1) * 128], wsg[:, c, :],
                            start=(c == 0), stop=False,
                        )
                    nc.tensor.matmul(ps_lg, ones_row[:, 0:128], bsg_row,
                                     start=False, stop=True)

                    # gumbel: lnz = ln(-ln(u+1e-8)+1e-8);  t = gamma*logits - lnz
                    lnu = pAs.tile([128, K], F32, tag="lnu")
                    nc.scalar.activation(lnu, gum, AF.Ln, bias=eps8_t)
                    lnz = pAs.tile([128, K], F32, tag="lnz")
                    nc.scalar.activation(lnz, lnu, AF.Ln, bias=eps8_t, scale=-1.0)
                    t_sb = pAs.tile([128, K], F32, tag="tsb")
                    nc.vector.scalar_tensor_tensor(
                        out=t_sb, in0=ps_lg[:, 0:K], scalar=gamma_t, in1=lnz,
                        op0=ALU.mult, op1=ALU.subtract,
                    )

                    # scores = sigmoid(gate) = 1/(1+exp(-gate))
                    sc_e = pAs.tile([128, 1], F32, tag="sce")
                    nc.scalar.activation(sc_e, ps_lg[:, K:K + 1], AF.Exp, scale=-1.0)
                    sc1 = pAs.tile([128, 1], F32, tag="sc1")
                    nc.vector.tensor_scalar_add(sc1, sc_e, 1.0)
                    scores = pAs.tile([128, 1], F32, tag="scores")
                    nc.vector.reciprocal(scores, sc1)

                    # p_unnorm = exp(t), row-sum fused; w = p_unnorm*(scores/rowsum)
                    p_un = pAs.tile([128, K], F32, tag="pun")
                    rs = pAs.tile([128, 1], F32, tag="rs")
                    nc.scalar.activation(p_un, t_sb, AF.Exp, accum_out=rs)
                    rrs = pAs.tile([128, 1], F32, tag="rrs")
                    nc.vector.reciprocal(rrs, rs)
                    s2 = pAs.tile([128, 1], F32, tag="s2")
                    nc.vector.tensor_tensor(s2, scores, rrs, ALU.mult)
                    w_sb = pAs.tile([128, K], F32R, tag="wsb")
                    nc.vector.tensor_scalar_mul(w_sb, p_un, s2)

                    # wv_sb = [ones | write_vals]
                    wv_sb = pAs.tile([128, DC + 2], F32R, tag="wvsb")
                    nc.vector.tensor_copy(out=wv_sb[:, 0:2], in_=ones_col2_f)
                    nc.vector.tensor_copy(out=wv_sb[:, 2:DC + 2], in_=ps_wv)

                    for kc in range(2):
                        lhs = w_sb[:, kc * 128:(kc + 1) * 128]
                        nc.tensor.matmul(ps_ua[kc], lhs, wv_sb[:, 0:K + 2],
                                         start=(i == 0), stop=(i == ST - 1))
                        nc.tensor.matmul(ps_ub[kc], lhs, wv_sb[:, K + 2:DC + 2],
                                         start=(i == 0), stop=(i == ST - 1))

                # ------- phase B: slot update, overwrite cache rows -------
                base_t = LAYER_IDX * K // 128  # n-tile 6
                for kc in range(2):
                    mass = pAs.tile([128, 1], F32, tag="mass")
                    nc.vector.tensor_copy(out=mass, in_=ps_ua[kc][:, 0:1])
                    m1 = pAs.tile([128, 1], F32, tag="m1")
                    nc.vector.tensor_scalar_add(m1, mass, EPS)
                    rm = pAs.tile([128, 1], F32, tag="rm")
                    nc.vector.reciprocal(rm, m1)
                    m2 = pAs.tile([128, 1], F32, tag="m2")
                    nc.vector.tensor_scalar_add(m2, mass, 1.0)
                    rg = pAs.tile([128, 1], F32, tag="rg")
                    nc.vector.reciprocal(rg, m2)
                    g_t = pAs.tile([128, 1], F32, tag="gt")
                    nc.vector.tensor_tensor(g_t, mass, rg, ALU.mult)
                    co = pAs.tile([128, 1], F32, tag="co")
                    nc.vector.tensor_scalar(co, g_t, -DECAY, DECAY, ALU.mult, ALU.add)
                    cn = pAs.tile([128, 1], F32, tag="cn")
                    nc.vector.tensor_tensor(cn, g_t, rm, ALU.mult)

                    told = pAs.tile([128, DC], F32, tag="told")
                    nc.vector.tensor_scalar_mul(told, cache_sb[:, base_t + kc, :], co)
                    nc.vector.scalar_tensor_tensor(
                        out=cache_sb[:, base_t + kc, 0:K],
                        in0=ps_ua[kc][:, 2:K + 2], scalar=cn, in1=told[:, 0:K],
                        op0=ALU.mult, op1=ALU.add,
                    )
                    nc.vector.scalar_tensor_tensor(
                        out=cache_sb[:, base_t + kc, K:DC],
                        in0=ps_ub[kc], scalar=cn, in1=told[:, K:DC],
                        op0=ALU.mult, op1=ALU.add,
                    )

            # ---------------- phases C-F ----------------------------------
            with (
                tc.tile_pool(name="woP", bufs=1) as woP,
                tc.tile_pool(name="aoP", bufs=1) as aoP,
            ):
                wo_sb = woP.tile([128, H, DC], BF16)
                nc.gpsimd.dma_start(out=wo_sb, in_=a["Wo"].rearrange("(c p) d -> p c d", p=128))
                aoT = aoP.tile([128, H, N], BF16)

                with (
                    tc.tile_pool(name="c2tP", bufs=1) as c2tP,
                    tc.tile_pool(name="wqkvP", bufs=1) as wqkvP,
                    tc.tile_pool(name="qkvP", bufs=1) as qkvP,
                ):
                    # ------- phase C: cache2 -> cache2T (bf16) -----------
                    c2t = c2tP.tile([128, 4, N], BF16)
                    with tc.tile_pool(name="psC", bufs=2, space="PSUM") as psC:
                        for j in range(4):
                            for tg in range(4):
                                ps = psC.tile([128, 512], F32, tag="ctr")
                                for tt in range(4):
                                    t = tg * 4 + tt
                                    nc.tensor.transpose(
                                        ps[:, tt * 128:(tt + 1) * 128],
                                        cache_sb[:, t, j * 128:(j + 1) * 128],
                                        ident,
                                    )
                                nc.vector.tensor_copy(
                                    out=c2t[:, j, tg * 512:(tg + 1) * 512], in_=ps)

                    # ------- phase D: q/k/v projections (bf16) -----------
                    wq_sb = wqkvP.tile([128, 4, DC], BF16)
                    nc.gpsimd.dma_start(out=wq_sb, in_=a["Wq"].rearrange("(c p) d -> p c d", p=128))
                    wk_sb = wqkvP.tile([128, 4, DC], BF16)
                    nc.gpsimd.dma_start(out=wk_sb, in_=a["Wk"].rearrange("(c p) d -> p c d", p=128))
                    wv_w = wqkvP.tile([128, 4, DC], BF16)
                    nc.gpsimd.dma_start(out=wv_w, in_=a["Wv"].rearrange("(c p) d -> p c d", p=128))

                    qT = qkvP.tile([128, H, N], BF16)
                    kT = qkvP.tile([128, H, N], BF16)
                    v_sb = qkvP.tile([128, NT, DC], BF16)

                    with tc.tile_pool(name="psD", bufs=3, space="PSUM") as psD:
                        for dst, w_t, b_t in ((qT, wq_sb, bqr), (kT, wk_sb, bkr)):
                            for h in range(H):
                                for c in range(4):
                                    ps = psD.tile([128, 512], F32, tag="qk")
                                    for j in range(4):
                                        nc.tensor.matmul(
                                            ps, w_t[:, j, h * 128:(h + 1) * 128],
                                            c2t[:, j, c * 512:(c + 1) * 512],
                                            start=(j == 0), stop=False,
                                        )
                                    nc.tensor.matmul(
                                        ps, b_t[:, h * 128:(h + 1) * 128],
                                        ones_row_bf[:, 0:512], start=False, stop=True,
                                    )
                                    nc.vector.tensor_copy(
                                        out=dst[:, h, c * 512:(c + 1) * 512], in_=ps)
                        for m in range(NT):
                            ps = psD.tile([128, DC], F32, tag="v")
                            for j in range(4):
                                nc.tensor.matmul(
                                    ps, c2t[:, j, m * 128:(m + 1) * 128], wv_w[:, j, :],
                                    start=(j == 0), stop=False,
                                )
                            nc.tensor.matmul(ps, ones_row_bf[:, 0:128], bvr,
                                             start=False, stop=True)
                            nc.vector.tensor_copy(out=v_sb[:, m, :], in_=ps)

                    # ------- phase E: attention (transposed layout) ------
                    with (
                        tc.tile_pool(name="pE", bufs=3) as pE,
                        tc.tile_pool(name="pEs", bufs=2) as pEs,
                        tc.tile_pool(name="psAtt", bufs=2, space="PSUM") as psAtt,
                        tc.tile_pool(name="psAo", bufs=1, space="PSUM") as psAo,
                        tc.tile_pool(name="psDen", bufs=1, space="PSUM") as psDen,
                        tc.tile_pool(name="psBc", bufs=1, space="PSUM") as psBc,
                    ):
                        ps_aoA = psAo.tile([128, 512], F32, tag="aoA")
                        ps_aoB = psAo.tile([128, 512], F32, tag="aoB")
                        for c in range(NCH):
                            ps_den = psDen.tile([128, CL], F32, tag="den")
                            for m in range(NT):
                                ps_a = psAtt.tile([128, H * CL], F32, tag="att")
                                for h in range(H):
                                    nc.tensor.matmul(
                                        ps_a[:, h * CL:(h + 1) * CL],
                                        kT[:, h, m * 128:(m + 1) * 128],
                                        qT[:, h, c * CL:(c + 1) * CL],
                                        start=True, stop=True,
                                    )
                                pT = pE.tile([128, H * CL], BF16, tag="pT")
                                nc.scalar.activation(pT, ps_a, AF.Exp, scale=ATT_SCALE)
                                for h in range(H):
                                    ps_ao = ps_aoA if h < 2 else ps_aoB
                                    # one accumulation group per PSUM bank: the
                                    # bank-wide zero region from the h-even start
                                    # makes the h-odd m==0 write an overwrite.
                                    nc.tensor.matmul(
                                        ps_ao[:, (h % 2) * CL:(h % 2 + 1) * CL],
                                        v_sb[:, m, h * 128:(h + 1) * 128],
                                        pT[:, h * CL:(h + 1) * CL],
                                        start=(m == 0 and h % 2 == 0),
                                        stop=(m == NT - 1 and h % 2 == 1),
                                    )
                                for h in range(H):
                                    nc.tensor.matmul(
                                        ps_den[32 * h:32 * h + 1, :],
                                        ones_col_bf,
                                        pT[:, h * CL:(h + 1) * CL],
                                        start=(m == 0), stop=(m == NT - 1),
                                        tile_position=(0, 32 * h),
                                    )
                            # denominators -> 1/den (via exp(-ln)) -> broadcast
                            den_sb = pEs.tile([128, CL], F32, tag="densb")
                            for h in range(H):
                                nc.vector.tensor_copy(
                                    out=den_sb[32 * h:32 * h + 1, :],
                                    in_=ps_den[32 * h:32 * h + 1, :],
                                )
                            den4 = pEs.tile([1, H * CL], F32, tag="den4")
                            for h in range(H):
                                nc.sync.dma_start(
                                    out=den4[:, h * CL:(h + 1) * CL],
                                    in_=den_sb[32 * h:32 * h + 1, :],
                                )
                            lnden = pEs.tile([1, H * CL], F32, tag="lnden")
                            nc.scalar.activation(lnden, den4, AF.Ln)
                            rden = pEs.tile([1, H * CL], F32R, tag="rden")
                            nc.scalar.activation(rden, lnden, AF.Exp, scale=-1.0)
                            for h in range(H):
                                bc = psBc.tile([128, CL], F32, tag="bc")
                                nc.tensor.matmul(
                                    bc,
                                    ones_row[:, 0:128],
                                    rden[:, h * CL:(h + 1) * CL],
                                    start=True, stop=True,
                                )
                                bc_sb = pEs.tile([128, CL], F32, tag="bcsb")
                                nc.vector.tensor_copy(out=bc_sb, in_=bc)
                                ps_ao = ps_aoA if h < 2 else ps_aoB
                                nc.vector.scalar_tensor_tensor(
                                    out=aoT[:, h, c * CL:(c + 1) * CL],
                                    in0=ps_ao[:, (h % 2) * CL:(h % 2 + 1) * CL],
                                    scalar=1.0, in1=bc_sb,
                                    op0=ALU.mult, op1=ALU.mult,
                                )

                # ------- phase F: o-proj + residual + layernorm ----------
                with (
                    tc.tile_pool(name="pF", bufs=2) as pF,
                    tc.tile_pool(name="psF", bufs=2, space="PSUM") as psF,
                ):
                    for t in range(NT):
                        ps_o = psF.tile([128, DC], F32, tag="o")
                        for h in range(H):
                            nc.tensor.matmul(
                                ps_o, aoT[:, h, t * 128:(t + 1) * 128], wo_sb[:, h, :],
                                start=(h == 0), stop=False,
                            )
                        nc.tensor.matmul(ps_o, ones_row_bf[:, 0:128], bor,
                                         start=False, stop=True)
                        r = pF.tile([128, DC], F32, tag="r")
                        rsum = pF.tile([128, 1], F32, tag="rsum")
                        nc.vector.scalar_tensor_tensor(
                            out=r, in0=ps_o, scalar=1.0, in1=cache_sb[:, t, :],
                            op0=ALU.mult, op1=ALU.add, accum_out=rsum,
                        )
                        mean = pF.tile([128, 1], F32, tag="mean")
                        nc.vector.tensor_scalar_mul(mean, rsum, 1.0 / DC)
                        scratch = pF.tile([128, DC], F32, tag="scratch")
                        ssq = pF.tile([128, 1], F32, tag="ssq")
                        nc.vector.scalar_tensor_tensor(
                            out=scratch, in0=r, scalar=mean, in1=r,
                            op0=ALU.subtract, op1=ALU.mult, accum_out=ssq,
                        )
                        lnv = pF.tile([128, 1], F32, tag="lnv")
                        nc.scalar.activation(lnv, ssq, AF.Ln, scale=1.0 / DC, bias=eps5_t)
                        rstd = pF.tile([128, 1], F32, tag="rstd")
                        nc.scalar.activation(rstd, lnv, AF.Exp, scale=-0.5)
                        t1 = pF.tile([128, DC], F32, tag="t1")
                        nc.vector.tensor_scalar(t1, r, mean, rstd, ALU.subtract, ALU.mult)
                        t2 = pF.tile([128, DC], F32, tag="t2")
                        nc.vector.scalar_tensor_tensor(
                            out=t2, in0=t1, scalar=1.0, in1=lng_bc,
                            op0=ALU.mult, op1=ALU.mult,
                        )
                        o_sb = pF.tile([128, DC], F32, tag="osb")
                        nc.vector.scalar_tensor_tensor(
                            out=o_sb, in0=t2, scalar=1.0, in1=lnb_bc,
                            op0=ALU.mult, op1=ALU.add,
                        )
                        nc.sync.dma_start(out=out3[:, t, :], in_=o_sb)

    nc.compile()
    return nc


_NC_CACHE = {}


def _get_nc():
    if "nc" not in _NC_CACHE:
        _NC_CACHE["nc"] = _build()
    return _NC_CACHE["nc"]


def _in_maps(inputs):
    per_batch = {"y", "cache", "gumbel_u"}
    maps = []
    for b in range(B):
        m = {}
        for name in _INPUT_SPECS:
            arr = np.ascontiguousarray(np.asarray(inputs[name], dtype=np.float32))
            m[name] = arr[b] if name in per_batch else arr
        maps.append(m)
    return maps


def _execute(inputs, trace=False):
    nc = _get_nc()
    res = run_bass_kernel_spmd(nc, _in_maps(inputs), list(range(B)), trace=trace)
    out = np.stack([res.results[b]["out"] for b in range(B)]).astype(np.float32)
    return out, res


def kernel(**inputs) -> np.ndarray:
    out, _ = _execute(inputs)
    return out


# revision 19
# speedup vs baseline: 1.3174x; 1.0262x over previous
"""DLSMN scatter-memory + cache self-attention kernel for Trainium2.

Data-parallel over batch: batch b runs on NeuronCore b (8 cores), no
collectives.  Inside one core (one batch):

  phase A: per 128-token tile of y: PE-transpose y -> yT chunks, fused
           matmuls  [W_write | (W_slot,W_gate)]  (fp32r), gumbel-softmax
           routing via exp(logits*gamma - ln(-ln(u+eps)+eps)) (single
           Ln/Exp ACT table set), weighted-scatter matmul with a leading
           ones column in the rhs so the write-mass comes out of the same
           accumulation for free.
  phase B: slot update  upd = (1-g)*DECAY*old + g*updates/(mass+eps).
  phase C: PE-transpose cache2 -> cache2T (bf16).
  phase D: q/k/v projections in bf16 (qT,kT transposed layout; v natural).
  phase E: attention computed transposed: attT[m,n] tiles; softmax has no
           max-subtraction (logits are provably tiny); denominators via
           col-tiled ones-matmuls (4 heads concurrent in the PE array);
           ao^T accumulated in PSUM; normalization by exp(-ln(den)).
  phase F: output projection + residual + layernorm (fused DVE
           scalar_tensor_tensor with accum_out row sums).
"""

import numpy as np

import concourse.bacc as bacc
import concourse.mybir as mybir
import concourse.tile as tile
from concourse.bass_utils import run_bass_kernel_spmd
from concourse.masks import make_identity

F32 = mybir.dt.float32
F32R = mybir.dt.float32r
BF16 = mybir.dt.bfloat16
AF = mybir.ActivationFunctionType
ALU = mybir.AluOpType

B = 8
S = 2048
D = 1024
DC = 512
K = 256
L = 8
H = 4
HD = 128
N = L * K
LAYER_IDX = 3
DECAY = 0.9
EPS = 1e-6
ST = S // 128  # 16 token tiles
NT = N // 128  # 16 slot tiles
DCH = D // 128  # 8 d_model chunks
CL = 256  # attention n-chunk length
NCH = N // CL  # 8 attention chunks
ATT_SCALE = float(1.0 / np.sqrt(np.float32(HD)))

_INPUT_SPECS = {
    "y": (S, D), "cache": (N, DC), "gumbel_u": (S, K),
    "W_gate": (D, 1), "b_gate": (1,), "W_slot": (D, K), "b_slot": (K,),
    "gamma": (1,), "W_write": (D, DC), "b_write": (DC,),
    "Wq": (DC, DC), "bq": (DC,), "Wk": (DC, DC), "bk": (DC,),
    "Wv": (DC, DC), "bv": (DC,), "Wo": (DC, DC), "bo": (DC,),
    "ln_g": (DC,), "ln_b": (DC,),
}


def _r(ap):
    return ap.bitcast(F32R)


def _build():
    nc = bacc.Bacc("TRN2", target_bir_lowering=False, debug=False, num_devices=B)

    a = {
        name: nc.dram_tensor(name, list(shape), F32, kind="ExternalInput").ap()
        for name, shape in _INPUT_SPECS.items()
    }
    out_dram = nc.dram_tensor("out", [N, DC], F32, kind="ExternalOutput").ap()

    y3 = a["y"].rearrange("(t p) d -> p t d", p=128)
    gum3 = a["gumbel_u"].rearrange("(t p) k -> p t k", p=128)
    cache3 = a["cache"].rearrange("(t p) d -> p t d", p=128)
    out3 = out_dram.rearrange("(t p) d -> p t d", p=128)

    with tile.TileContext(nc) as tc:
        with (
            tc.tile_pool(name="const", bufs=1) as const,
            tc.tile_pool(name="cachep", bufs=1) as cachep,
        ):
            ident = const.tile([128, 128], F32)
            make_identity(nc, ident)
            ones_row_f = const.tile([1, DC], F32)
            nc.vector.memset(ones_row_f, 1.0)
            ones_col2_f = const.tile([128, 2], F32)
            nc.vector.memset(ones_col2_f, 1.0)
            ones_row = const.tile([1, DC], F32R)
            nc.vector.tensor_copy(out=ones_row, in_=ones_row_f)
            ones_row_bf = const.tile([1, DC], BF16)
            nc.vector.memset(ones_row_bf, 1.0)
            ones_col_bf = const.tile([128, 1], BF16)
            nc.vector.memset(ones_col_bf, 1.0)
            eps8_t = const.tile([128, 1], F32)
            nc.vector.memset(eps8_t, 1e-8)
            eps5_t = const.tile([128, 1], F32)
            nc.vector.memset(eps5_t, 1e-5)
            gamma_t = const.tile([128, 1], F32)
            nc.sync.dma_start(out=gamma_t, in_=a["gamma"].unsqueeze(0).to_broadcast([128, 1]))
            lng_bc = const.tile([128, DC], F32)
            nc.sync.dma_start(out=lng_bc, in_=a["ln_g"].unsqueeze(0).to_broadcast([128, DC]))
            lnb_bc = const.tile([128, DC], F32)
            nc.sync.dma_start(out=lnb_bc, in_=a["ln_b"].unsqueeze(0).to_broadcast([128, DC]))
            bwr_row = const.tile([1, DC], F32R)
            nc.gpsimd.dma_start(out=bwr_row, in_=a["b_write"].unsqueeze(0))
            bsg_row = const.tile([1, K + 2], F32R)
            nc.gpsimd.dma_start(out=bsg_row[:, 0:K], in_=a["b_slot"].unsqueeze(0))
            nc.gpsimd.dma_start(out=bsg_row[:, K:K + 1], in_=a["b_gate"].unsqueeze(0))
            nc.gpsimd.dma_start(out=bsg_row[:, K + 1:K + 2], in_=a["b_gate"].unsqueeze(0))
            # bf16 bias rows for the attention-side projections
            bqr = const.tile([1, DC], BF16)
            nc.gpsimd.dma_start(out=bqr, in_=a["bq"].unsqueeze(0))
            bkr = const.tile([1, DC], BF16)
            nc.gpsimd.dma_start(out=bkr, in_=a["bk"].unsqueeze(0))
            bvr = const.tile([1, DC], BF16)
            nc.gpsimd.dma_start(out=bvr, in_=a["bv"].unsqueeze(0))
            bor = const.tile([1, DC], BF16)
            nc.gpsimd.dma_start(out=bor, in_=a["bo"].unsqueeze(0))

            cache_sb = cachep.tile([128, NT, DC], F32)

            # ---------------- phase A + B: selection & scatter write ------
            with (
                tc.tile_pool(name="wA", bufs=1) as wA,
                tc.tile_pool(name="pA", bufs=2) as pA,
                tc.tile_pool(name="pAs", bufs=3) as pAs,
                tc.tile_pool(name="psU", bufs=1, space="PSUM") as psU,
                tc.tile_pool(name="psA", bufs=1, space="PSUM") as psA,
                tc.tile_pool(name="psT", bufs=2, space="PSUM") as psT,
            ):
                wwr = wA.tile([128, DCH, DC], F32R)
                wsg = wA.tile([128, DCH, K + 2], F32R)

                # gumbel pre-pass: all Ln ops batched (one ACT table residency)
                lnz_all = wA.tile([128, ST, K], F32)
                for i in range(ST):
                    gum = pA.tile([128, K], F32, tag="gum")
                    nc.sync.dma_start(out=gum, in_=gum3[:, i, :])
                    lnu = pAs.tile([128, K], F32, tag="lnu")
                    nc.scalar.activation(lnu, gum, AF.Ln, bias=eps8_t)
                    nc.scalar.activation(lnz_all[:, i, :], lnu, AF.Ln, bias=eps8_t,
                                         scale=-1.0)

                # persistent scatter accumulators: [ones|wv] x w  ->  [mass | updates]
                ps_ua = [psU.tile([128, K + 2], F32, name=f"ua{kc}", tag=f"ua{kc}")
                         for kc in range(2)]
                ps_ub = [psU.tile([128, K], F32, name=f"ub{kc}", tag=f"ub{kc}")
                         for kc in range(2)]

                for i in range(ST):
                    y_t = pA.tile([128, D], F32, tag="y")
                    nc.sync.dma_start(out=y_t, in_=y3[:, i, :])
                    if i == 0:
                        nc.gpsimd.dma_start(out=wwr, in_=a["W_write"].rearrange("(c p) d -> p c d", p=128))
                        nc.gpsimd.dma_start(out=wsg[:, :, 0:K], in_=a["W_slot"].rearrange("(c p) k -> p c k", p=128))
                        nc.gpsimd.dma_start(out=wsg[:, :, K:K + 1], in_=a["W_gate"].rearrange("(c p) o -> p c o", p=128))
                        nc.gpsimd.dma_start(out=wsg[:, :, K + 1:K + 2], in_=a["W_gate"].rearrange("(c p) o -> p c o", p=128))
                        nc.sync.dma_start(out=cache_sb, in_=cache3)

                    # transpose y tile -> yT (8 chunks of [128d, 128s])
                    yT = pA.tile([128, D], F32R, tag="yT")
                    for g in range(2):
                        tr = psT.tile([128, 512], F32, tag="tr")
                        for cc in range(4):
                            c = 4 * g + cc
                            nc.tensor.transpose(
                                tr[:, cc * 128:(cc + 1) * 128],
                                y_t[:, c * 128:(c + 1) * 128],
                                ident,
                            )
                        nc.vector.tensor_copy(out=yT[:, g * 512:(g + 1) * 512], in_=tr)

                    # fused write_vals / (logits, gate) matmuls
                    ps_wv = psA.tile([128, DC], F32, tag="wv")
                    for c in range(DCH):
                        nc.tensor.matmul(
                            ps_wv, yT[:, c * 128:(c + 1) * 128], wwr[:, c, :],
                            start=(c == 0), stop=False,
                        )
                    nc.tensor.matmul(ps_wv, ones_row[:, 0:128], bwr_row,
                                     start=False, stop=True)
                    ps_lg = psA.tile([128, K + 2], F32, tag="lg")
                    for c in range(DCH):
                        nc.tensor.matmul(
                            ps_lg, yT[:, c * 128:(c + 1) * 128], wsg[:, c, :],
                            start=(c == 0), stop=False,
                        )
                    nc.tensor.matmul(ps_lg, ones_row[:, 0:128], bsg_row,
                                     start=False, stop=True)

                    # t = gamma*logits - lnz   (lnz precomputed in the pre-pass)
                    t_sb = pAs.tile([128, K], F32, tag="tsb")
                    nc.vector.scalar_tensor_tensor(
                        out=t_sb, in0=ps_lg[:, 0:K], scalar=gamma_t, in1=lnz_all[:, i, :],
                        op0=ALU.mult, op1=ALU.subtract,
                    )

                    # scores = sigmoid(gate) = 1/(1+exp(-gate))
                    sc_e = pAs.tile([128, 1], F32, tag="sce")
                    nc.scalar.activation(sc_e, ps_lg[:, K:K + 1], AF.Exp, scale=-1.0)
                    sc1 = pAs.tile([128, 1], F32, tag="sc1")
                    nc.vector.tensor_scalar_add(sc1, sc_e, 1.0)
                    scores = pAs.tile([128, 1], F32, tag="scores")
                    nc.vector.reciprocal(scores, sc1)

                    # p_unnorm = exp(t), row-sum fused; w = p_unnorm*(scores/rowsum)
                    p_un = pAs.tile([128, K], F32, tag="pun")
                    rs = pAs.tile([128, 1], F32, tag="rs")
                    nc.scalar.activation(p_un, t_sb, AF.Exp, accum_out=rs)
                    rrs = pAs.tile([128, 1], F32, tag="rrs")
                    nc.vector.reciprocal(rrs, rs)
                    s2 = pAs.tile([128, 1], F32, tag="s2")
                    nc.vector.tensor_tensor(s2, scores, rrs, ALU.mult)
                    w_sb = pAs.tile([128, K], F32R, tag="wsb")
                    nc.vector.tensor_scalar_mul(w_sb, p_un, s2)

                    # wv_sb = [ones | write_vals]
                    wv_sb = pAs.tile([128, DC + 2], F32R, tag="wvsb")
                    nc.vector.tensor_copy(out=wv_sb[:, 0:2], in_=ones_col2_f)
                    nc.vector.tensor_copy(out=wv_sb[:, 2:DC + 2], in_=ps_wv)

                    for kc in range(2):
                        lhs = w_sb[:, kc * 128:(kc + 1) * 128]
                        nc.tensor.matmul(ps_ua[kc], lhs, wv_sb[:, 0:K + 2],
                                         start=(i == 0), stop=(i == ST - 1))
                        nc.tensor.matmul(ps_ub[kc], lhs, wv_sb[:, K + 2:DC + 2],
                                         start=(i == 0), stop=(i == ST - 1))

                # ------- phase B: slot update, overwrite cache rows -------
                base_t = LAYER_IDX * K // 128  # n-tile 6
                for kc in range(2):
                    mass = pAs.tile([128, 1], F32, tag="mass")
                    nc.vector.tensor_copy(out=mass, in_=ps_ua[kc][:, 0:1])
                    m1 = pAs.tile([128, 1], F32, tag="m1")
                    nc.vector.tensor_scalar_add(m1, mass, EPS)
                    rm = pAs.tile([128, 1], F32, tag="rm")
                    nc.vector.reciprocal(rm, m1)
                    m2 = pAs.tile([128, 1], F32, tag="m2")
                    nc.vector.tensor_scalar_add(m2, mass, 1.0)
                    rg = pAs.tile([128, 1], F32, tag="rg")
                    nc.vector.reciprocal(rg, m2)
                    g_t = pAs.tile([128, 1], F32, tag="gt")
                    nc.vector.tensor_tensor(g_t, mass, rg, ALU.mult)
                    co = pAs.tile([128, 1], F32, tag="co")
                    nc.vector.tensor_scalar(co, g_t, -DECAY, DECAY, ALU.mult, ALU.add)
                    cn = pAs.tile([128, 1], F32, tag="cn")
                    nc.vector.tensor_tensor(cn, g_t, rm, ALU.mult)

                    told = pAs.tile([128, DC], F32, tag="told")
                    nc.vector.tensor_scalar_mul(told, cache_sb[:, base_t + kc, :], co)
                    nc.vector.scalar_tensor_tensor(
                        out=cache_sb[:, base_t + kc, 0:K],
                        in0=ps_ua[kc][:, 2:K + 2], scalar=cn, in1=told[:, 0:K],
                        op0=ALU.mult, op1=ALU.add,
                    )
                    nc.vector.scalar_tensor_tensor(
                        out=cache_sb[:, base_t + kc, K:DC],
                        in0=ps_ub[kc], scalar=cn, in1=told[:, K:DC],
                        op0=ALU.mult, op1=ALU.add,
                    )

            # ---------------- phases C-F ----------------------------------
            with (
                tc.tile_pool(name="woP", bufs=1) as woP,
                tc.tile_pool(name="aoP", bufs=1) as aoP,
            ):
                wo_sb = woP.tile([128, H, DC], BF16)
                nc.gpsimd.dma_start(out=wo_sb, in_=a["Wo"].rearrange("(c p) d -> p c d", p=128))
                aoT = aoP.tile([128, H, N], BF16)

                with (
                    tc.tile_pool(name="c2tP", bufs=1) as c2tP,
                    tc.tile_pool(name="wqkvP", bufs=1) as wqkvP,
                    tc.tile_pool(name="qkvP", bufs=1) as qkvP,
                ):
                    # ------- phase C: cache2 -> cache2T (bf16) -----------
                    c2t = c2tP.tile([128, 4, N], BF16)
                    with tc.tile_pool(name="psC", bufs=2, space="PSUM") as psC:
                        for j in range(4):
                            for tg in range(4):
                                ps = psC.tile([128, 512], F32, tag="ctr")
                                for tt in range(4):
                                    t = tg * 4 + tt
                                    nc.tensor.transpose(
                                        ps[:, tt * 128:(tt + 1) * 128],
                                        cache_sb[:, t, j * 128:(j + 1) * 128],
                                        ident,
                                    )
                                nc.scalar.copy(
                                    out=c2t[:, j, tg * 512:(tg + 1) * 512], in_=ps)

                    # ------- phase D: q/k/v projections (bf16) -----------
                    wq_sb = wqkvP.tile([128, 4, DC], BF16)
                    nc.gpsimd.dma_start(out=wq_sb, in_=a["Wq"].rearrange("(c p) d -> p c d", p=128))
                    wk_sb = wqkvP.tile([128, 4, DC], BF16)
                    nc.gpsimd.dma_start(out=wk_sb, in_=a["Wk"].rearrange("(c p) d -> p c d", p=128))
                    wv_w = wqkvP.tile([128, 4, DC], BF16)
                    nc.gpsimd.dma_start(out=wv_w, in_=a["Wv"].rearrange("(c p) d -> p c d", p=128))

                    qT = qkvP.tile([128, H, N], BF16)
                    kT = qkvP.tile([128, H, N], BF16)
                    v_sb = qkvP.tile([128, NT, DC], BF16)

                    with tc.tile_pool(name="psD", bufs=3, space="PSUM") as psD:
                        for dst, w_t, b_t in ((qT, wq_sb, bqr), (kT, wk_sb, bkr)):
                            for h in range(H):
                                for c in range(4):
                                    ps = psD.tile([128, 512], F32, tag="qk")
                                    for j in range(4):
                                        nc.tensor.matmul(
                                            ps, w_t[:, j, h * 128:(h + 1) * 128],
                                            c2t[:, j, c * 512:(c + 1) * 512],
                                            start=(j == 0), stop=False,
                                        )
                                    nc.tensor.matmul(
                                        ps, b_t[:, h * 128:(h + 1) * 128],
                                        ones_row_bf[:, 0:512], start=False, stop=True,
                                    )
                                    nc.scalar.copy(
                                        out=dst[:, h, c * 512:(c + 1) * 512], in_=ps)
                        for m in range(NT):
                            ps = psD.tile([128, DC], F32, tag="v")
                            for j in range(4):
                                nc.tensor.matmul(
                                    ps, c2t[:, j, m * 128:(m + 1) * 128], wv_w[:, j, :],
                                    start=(j == 0), stop=False,
                                )
                            nc.tensor.matmul(ps, ones_row_bf[:, 0:128], bvr,
                                             start=False, stop=True)
                            nc.scalar.copy(out=v_sb[:, m, :], in_=ps)

                    # ------- phase E: attention (transposed layout) ------
                    with (
                        tc.tile_pool(name="pE", bufs=3) as pE,
                        tc.tile_pool(name="pEs", bufs=2) as pEs,
                        tc.tile_pool(name="psAtt", bufs=2, space="PSUM") as psAtt,
                        tc.tile_pool(name="psAo", bufs=1, space="PSUM") as psAo,
                        tc.tile_pool(name="psDen", bufs=2, space="PSUM") as psDen,
                    ):
                        ps_aoA = psAo.tile([128, 512], F32, tag="aoA")
                        ps_aoB = psAo.tile([128, 512], F32, tag="aoB")
                        for c in range(NCH):
                            ps_den = psDen.tile([128, CL], F32, tag="den")
                            for m in range(NT):
                                ps_a = psAtt.tile([128, H * CL], F32, tag="att")
                                for h in range(H):
                                    nc.tensor.matmul(
                                        ps_a[:, h * CL:(h + 1) * CL],
                                        kT[:, h, m * 128:(m + 1) * 128],
                                        qT[:, h, c * CL:(c + 1) * CL],
                                        start=True, stop=True,
                                    )
                                pT = pE.tile([128, H * CL], BF16, tag="pT")
                                nc.scalar.activation(pT, ps_a, AF.Exp, scale=ATT_SCALE)
                                for h in range(H):
                                    ps_ao = ps_aoA if h < 2 else ps_aoB
                                    # one accumulation group per PSUM bank: the
                                    # bank-wide zero region from the h-even start
                                    # makes the h-odd m==0 write an overwrite.
                                    nc.tensor.matmul(
                                        ps_ao[:, (h % 2) * CL:(h % 2 + 1) * CL],
                                        v_sb[:, m, h * 128:(h + 1) * 128],
                                        pT[:, h * CL:(h + 1) * CL],
                                        start=(m == 0 and h % 2 == 0),
                                        stop=(m == NT - 1 and h % 2 == 1),
                                    )
                                for h in range(H):
                                    nc.tensor.matmul(
                                        ps_den[32 * h:32 * h + 1, :],
                                        ones_col_bf,
                                        pT[:, h * CL:(h + 1) * CL],
                                        start=(m == 0), stop=(m == NT - 1),
                                        tile_position=(0, 32 * h),
                                    )
                            # free the ao banks fast: unnormalized copy to SBUF
                            aoU = pEs.tile([128, H * CL], F32, tag="aoU")
                            nc.vector.tensor_copy(out=aoU[:, 0:2 * CL], in_=ps_aoA)
                            nc.vector.tensor_copy(out=aoU[:, 2 * CL:4 * CL], in_=ps_aoB)
                            # denominators -> 1/den on DVE (keeps the Exp table resident)
                            den_sb = pEs.tile([128, CL], F32, tag="densb")
                            nc.vector.memset(den_sb, 1.0)
                            for h in range(H):
                                nc.vector.tensor_copy(
                                    out=den_sb[32 * h:32 * h + 1, :],
                                    in_=ps_den[32 * h:32 * h + 1, :],
                                )
                            rden_sb = pEs.tile([128, CL], F32R, tag="rdensb")
                            with nc.allow_low_precision(reason="1/den stored f32r for the PE broadcast"):
                                nc.vector.reciprocal(rden_sb, den_sb)
                            rden = pEs.tile([1, H * CL], F32R, tag="rden")
                            for h in range(H):
                                nc.sync.dma_start(
                                    out=rden[:, h * CL:(h + 1) * CL],
                                    in_=rden_sb[32 * h:32 * h + 1, :],
                                )
                            bc = psAtt.tile([128, H * CL], F32, tag="att")
                            for h in range(H):
                                nc.tensor.matmul(
                                    bc[:, h * CL:(h + 1) * CL],
                                    ones_row[:, 0:128],
                                    rden[:, h * CL:(h + 1) * CL],
                                    start=(h % 2 == 0), stop=(h % 2 == 1),
                                )
                            bc_sb = pEs.tile([128, H * CL], F32, tag="bcsb")
                            nc.vector.tensor_copy(out=bc_sb, in_=bc)
                            for h in range(H):
                                nc.vector.scalar_tensor_tensor(
                                    out=aoT[:, h, c * CL:(c + 1) * CL],
                                    in0=aoU[:, h * CL:(h + 1) * CL],
                                    scalar=1.0, in1=bc_sb[:, h * CL:(h + 1) * CL],
                                    op0=ALU.mult, op1=ALU.mult,
                                )

                # ------- phase F: o-proj + residual + layernorm ----------
                with (
                    tc.tile_pool(name="pF", bufs=2) as pF,
                    tc.tile_pool(name="pFbig", bufs=1) as pFbig,
                    tc.tile_pool(name="psF", bufs=2, space="PSUM") as psF,
                ):
                    r_all = pFbig.tile([128, NT, DC], F32)
                    mean_all = pFbig.tile([128, NT], F32)
                    ssq_all = pFbig.tile([128, NT], F32)
                    for t in range(NT):
                        ps_o = psF.tile([128, DC], F32, tag="o")
                        for h in range(H):
                            nc.tensor.matmul(
                                ps_o, aoT[:, h, t * 128:(t + 1) * 128], wo_sb[:, h, :],
                                start=(h == 0), stop=False,
                            )
                        nc.tensor.matmul(ps_o, ones_row_bf[:, 0:128], bor,
                                         start=False, stop=True)
                        rsum = pF.tile([128, 1], F32, tag="rsum")
                        nc.vector.scalar_tensor_tensor(
                            out=r_all[:, t, :], in0=ps_o, scalar=1.0,
                            in1=cache_sb[:, t, :],
                            op0=ALU.mult, op1=ALU.add, accum_out=rsum,
                        )
                        nc.vector.tensor_scalar_mul(
                            mean_all[:, t:t + 1], rsum, 1.0 / DC)
                        scratch = pF.tile([128, DC], F32, tag="scratch")
                        nc.vector.scalar_tensor_tensor(
                            out=scratch, in0=r_all[:, t, :],
                            scalar=mean_all[:, t:t + 1], in1=r_all[:, t, :],
                            op0=ALU.subtract, op1=ALU.mult,
                            accum_out=ssq_all[:, t:t + 1],
                        )
                    lnv_all = pFbig.tile([128, NT], F32)
                    nc.scalar.activation(lnv_all, ssq_all, AF.Ln, scale=1.0 / DC,
                                         bias=eps5_t)
                    rstd_all = pFbig.tile([128, NT], F32)
                    nc.scalar.activation(rstd_all, lnv_all, AF.Exp, scale=-0.5)
                    for t in range(NT):
                        t1 = pF.tile([128, DC], F32, tag="t1")
                        nc.vector.tensor_scalar(
                            t1, r_all[:, t, :], mean_all[:, t:t + 1],
                            rstd_all[:, t:t + 1], ALU.subtract, ALU.mult)
                        t2 = pF.tile([128, DC], F32, tag="t2")
                        nc.vector.scalar_tensor_tensor(
                            out=t2, in0=t1, scalar=1.0, in1=lng_bc,
                            op0=ALU.mult, op1=ALU.mult,
                        )
                        o_sb = pF.tile([128, DC], F32, tag="osb")
                        nc.vector.scalar_tensor_tensor(
                            out=o_sb, in0=t2, scalar=1.0, in1=lnb_bc,
                            op0=ALU.mult, op1=ALU.add,
                        )
                        nc.sync.dma_start(out=out3[:, t, :], in_=o_sb)

    nc.compile()
    return nc


_NC_CACHE = {}


def _get_nc():
    if "nc" not in _NC_CACHE:
        _NC_CACHE["nc"] = _build()
    return _NC_CACHE["nc"]


def _in_maps(inputs):
    per_batch = {"y", "cache", "gumbel_u"}
    maps = []
    for b in range(B):
        m = {}
        for name in _INPUT_SPECS:
            arr = np.ascontiguousarray(np.asarray(inputs[name], dtype=np.float32))
            m[name] = arr[b] if name in per_batch else arr
        maps.append(m)
    return maps


def _execute(inputs, trace=False):
    nc = _get_nc()
    res = run_bass_kernel_spmd(nc, _in_maps(inputs), list(range(B)), trace=trace)
    out = np.stack([res.results[b]["out"] for b in range(B)]).astype(np.float32)
    return out, res


def kernel(**inputs) -> np.ndarray:
    out, _ = _execute(inputs)
    return out


# revision 20
# speedup vs baseline: 1.5298x; 1.1613x over previous
"""DLSMN scatter-memory + cache self-attention kernel for Trainium2.

Data-parallel over batch: batch b runs on NeuronCore b (8 cores), no
collectives.  Inside one core (one batch):

  phase A: per 128-token tile of y: PE-transpose y -> yT chunks, fused
           matmuls  [W_write | (W_slot,W_gate)]  (fp32r), gumbel-softmax
           routing via exp(logits*gamma - ln(-ln(u+eps)+eps)) (single
           Ln/Exp ACT table set), weighted-scatter matmul with a leading
           ones column in the rhs so the write-mass comes out of the same
           accumulation for free.
  phase B: slot update  upd = (1-g)*DECAY*old + g*updates/(mass+eps).
  phase C: PE-transpose cache2 -> cache2T (bf16).
  phase D: q/k/v projections in bf16 (qT,kT transposed layout; v natural).
  phase E: attention computed transposed: attT[m,n] tiles; softmax has no
           max-subtraction (logits are provably tiny); denominators via
           col-tiled ones-matmuls (4 heads concurrent in the PE array);
           ao^T accumulated in PSUM; normalization by exp(-ln(den)).
  phase F: output projection + residual + layernorm (fused DVE
           scalar_tensor_tensor with accum_out row sums).
"""

import numpy as np

import concourse.bacc as bacc
import concourse.mybir as mybir
import concourse.tile as tile
from concourse.bass_utils import run_bass_kernel_spmd
from concourse.masks import make_identity

F32 = mybir.dt.float32
F32R = mybir.dt.float32r
BF16 = mybir.dt.bfloat16
AF = mybir.ActivationFunctionType
ALU = mybir.AluOpType

B = 8
S = 2048
D = 1024
DC = 512
K = 256
L = 8
H = 4
HD = 128
N = L * K
LAYER_IDX = 3
DECAY = 0.9
EPS = 1e-6
ST = S // 128  # 16 token tiles
NT = N // 128  # 16 slot tiles
DCH = D // 128  # 8 d_model chunks
CL = 256  # attention n-chunk length
NCH = N // CL  # 8 attention chunks
ATT_SCALE = float(1.0 / np.sqrt(np.float32(HD)))

_INPUT_SPECS = {
    "y": (S, D), "cache": (N, DC), "gumbel_u": (S, K),
    "W_gate": (D, 1), "b_gate": (1,), "W_slot": (D, K), "b_slot": (K,),
    "gamma": (1,), "W_write": (D, DC), "b_write": (DC,),
    "Wq": (DC, DC), "bq": (DC,), "Wk": (DC, DC), "bk": (DC,),
    "Wv": (DC, DC), "bv": (DC,), "Wo": (DC, DC), "bo": (DC,),
    "ln_g": (DC,), "ln_b": (DC,),
}


def _r(ap):
    return ap.bitcast(F32R)


def _build():
    nc = bacc.Bacc("TRN2", target_bir_lowering=False, debug=False, num_devices=B)

    a = {
        name: nc.dram_tensor(name, list(shape), F32, kind="ExternalInput").ap()
        for name, shape in _INPUT_SPECS.items()
    }
    out_dram = nc.dram_tensor("out", [N, DC], F32, kind="ExternalOutput").ap()

    y3 = a["y"].rearrange("(t p) d -> p t d", p=128)
    gum3 = a["gumbel_u"].rearrange("(t p) k -> p t k", p=128)
    cache3 = a["cache"].rearrange("(t p) d -> p t d", p=128)
    out3 = out_dram.rearrange("(t p) d -> p t d", p=128)

    with tile.TileContext(nc) as tc:
        with (
            tc.tile_pool(name="const", bufs=1) as const,
            tc.tile_pool(name="cachep", bufs=1) as cachep,
        ):
            ident = const.tile([128, 128], F32)
            make_identity(nc, ident)
            ones_row_f = const.tile([1, DC], F32)
            nc.vector.memset(ones_row_f, 1.0)
            ones_col2_f = const.tile([128, 2], F32)
            nc.vector.memset(ones_col2_f, 1.0)
            ones_row = const.tile([1, DC], F32R)
            nc.vector.tensor_copy(out=ones_row, in_=ones_row_f)
            ones_row_bf = const.tile([1, DC], BF16)
            nc.vector.memset(ones_row_bf, 1.0)
            ones_col_bf = const.tile([128, 1], BF16)
            nc.vector.memset(ones_col_bf, 1.0)
            eps8_t = const.tile([128, 1], F32)
            nc.vector.memset(eps8_t, 1e-8)
            eps5_t = const.tile([128, 1], F32)
            nc.vector.memset(eps5_t, 1e-5)
            gamma_t = const.tile([128, 1], F32)
            nc.sync.dma_start(out=gamma_t, in_=a["gamma"].unsqueeze(0).to_broadcast([128, 1]))
            lng_bc = const.tile([128, DC], F32)
            nc.sync.dma_start(out=lng_bc, in_=a["ln_g"].unsqueeze(0).to_broadcast([128, DC]))
            lnb_bc = const.tile([128, DC], F32)
            nc.sync.dma_start(out=lnb_bc, in_=a["ln_b"].unsqueeze(0).to_broadcast([128, DC]))
            bwr_row = const.tile([1, DC], F32R)
            nc.gpsimd.dma_start(out=bwr_row, in_=a["b_write"].unsqueeze(0))
            bsg_row = const.tile([1, K + 2], F32R)
            nc.gpsimd.dma_start(out=bsg_row[:, 0:K], in_=a["b_slot"].unsqueeze(0))
            nc.gpsimd.dma_start(out=bsg_row[:, K:K + 1], in_=a["b_gate"].unsqueeze(0))
            nc.gpsimd.dma_start(out=bsg_row[:, K + 1:K + 2], in_=a["b_gate"].unsqueeze(0))
            # bf16 bias rows for the attention-side projections
            bqr = const.tile([1, DC], BF16)
            nc.gpsimd.dma_start(out=bqr, in_=a["bq"].unsqueeze(0))
            bkr = const.tile([1, DC], BF16)
            nc.gpsimd.dma_start(out=bkr, in_=a["bk"].unsqueeze(0))
            bvr = const.tile([1, DC], BF16)
            nc.gpsimd.dma_start(out=bvr, in_=a["bv"].unsqueeze(0))
            bor = const.tile([1, DC], BF16)
            nc.gpsimd.dma_start(out=bor, in_=a["bo"].unsqueeze(0))

            cache_sb = cachep.tile([128, NT, DC], F32)

            # ---------------- phase A + B: selection & scatter write ------
            with (
                tc.tile_pool(name="wA", bufs=1) as wA,
                tc.tile_pool(name="pA", bufs=2) as pA,
                tc.tile_pool(name="pAs", bufs=3) as pAs,
                tc.tile_pool(name="psU", bufs=1, space="PSUM") as psU,
                tc.tile_pool(name="psA", bufs=1, space="PSUM") as psA,
                tc.tile_pool(name="psT", bufs=2, space="PSUM") as psT,
            ):
                wwr = wA.tile([128, DCH, DC], F32R)
                wsg = wA.tile([128, DCH, K + 2], F32R)

                # gumbel pre-pass: all Ln ops batched (one ACT table residency)
                lnz_all = wA.tile([128, ST, K], F32)
                for i in range(ST):
                    gum = pA.tile([128, K], F32, tag="gum")
                    nc.sync.dma_start(out=gum, in_=gum3[:, i, :])
                    lnu = pAs.tile([128, K], F32, tag="lnu")
                    nc.scalar.activation(lnu, gum, AF.Ln, bias=eps8_t)
                    nc.scalar.activation(lnz_all[:, i, :], lnu, AF.Ln, bias=eps8_t,
                                         scale=-1.0)

                # persistent scatter accumulators: [ones|wv] x w  ->  [mass | updates]
                ps_ua = [psU.tile([128, K + 2], F32, name=f"ua{kc}", tag=f"ua{kc}")
                         for kc in range(2)]
                ps_ub = [psU.tile([128, K], F32, name=f"ub{kc}", tag=f"ub{kc}")
                         for kc in range(2)]

                pending = []

                def flush_updates():
                    while pending:
                        j, w_j, wv_j = pending.pop(0)
                        for kc in range(2):
                            lhs = w_j[:, kc * 128:(kc + 1) * 128]
                            nc.tensor.matmul(ps_ua[kc], lhs, wv_j[:, 0:K + 2],
                                             start=(j == 0), stop=(j == ST - 1))
                            nc.tensor.matmul(ps_ub[kc], lhs, wv_j[:, K + 2:DC + 2],
                                             start=(j == 0), stop=(j == ST - 1))

                for i in range(ST):
                    y_t = pA.tile([128, D], F32, tag="y")
                    nc.sync.dma_start(out=y_t, in_=y3[:, i, :])
                    if i == 0:
                        nc.gpsimd.dma_start(out=wwr, in_=a["W_write"].rearrange("(c p) d -> p c d", p=128))
                        nc.gpsimd.dma_start(out=wsg[:, :, 0:K], in_=a["W_slot"].rearrange("(c p) k -> p c k", p=128))
                        nc.gpsimd.dma_start(out=wsg[:, :, K:K + 1], in_=a["W_gate"].rearrange("(c p) o -> p c o", p=128))
                        nc.gpsimd.dma_start(out=wsg[:, :, K + 1:K + 2], in_=a["W_gate"].rearrange("(c p) o -> p c o", p=128))
                        nc.sync.dma_start(out=cache_sb, in_=cache3)

                    # transpose y tile -> yT (8 chunks of [128d, 128s])
                    yT = pA.tile([128, D], F32R, tag="yT")
                    for g in range(2):
                        tr = psT.tile([128, 512], F32, tag="tr")
                        for cc in range(4):
                            c = 4 * g + cc
                            nc.tensor.transpose(
                                tr[:, cc * 128:(cc + 1) * 128],
                                y_t[:, c * 128:(c + 1) * 128],
                                ident,
                            )
                        nc.vector.tensor_copy(out=yT[:, g * 512:(g + 1) * 512], in_=tr)
                    flush_updates()

                    # fused write_vals / (logits, gate) matmuls
                    ps_wv = psA.tile([128, DC], F32, tag="wv")
                    for c in range(DCH):
                        nc.tensor.matmul(
                            ps_wv, yT[:, c * 128:(c + 1) * 128], wwr[:, c, :],
                            start=(c == 0), stop=False,
                        )
                    nc.tensor.matmul(ps_wv, ones_row[:, 0:128], bwr_row,
                                     start=False, stop=True)
                    ps_lg = psA.tile([128, K + 2], F32, tag="lg")
                    for c in range(DCH):
                        nc.tensor.matmul(
                            ps_lg, yT[:, c * 128:(c + 1) * 128], wsg[:, c, :],
                            start=(c == 0), stop=False,
                        )
                    nc.tensor.matmul(ps_lg, ones_row[:, 0:128], bsg_row,
                                     start=False, stop=True)

                    # t = gamma*logits - lnz   (lnz precomputed in the pre-pass)
                    t_sb = pAs.tile([128, K], F32, tag="tsb")
                    nc.vector.scalar_tensor_tensor(
                        out=t_sb, in0=ps_lg[:, 0:K], scalar=gamma_t, in1=lnz_all[:, i, :],
                        op0=ALU.mult, op1=ALU.subtract,
                    )

                    # scores = sigmoid(gate) = 1/(1+exp(-gate))
                    sc_e = pAs.tile([128, 1], F32, tag="sce")
                    nc.scalar.activation(sc_e, ps_lg[:, K:K + 1], AF.Exp, scale=-1.0)
                    sc1 = pAs.tile([128, 1], F32, tag="sc1")
                    nc.vector.tensor_scalar_add(sc1, sc_e, 1.0)
                    scores = pAs.tile([128, 1], F32, tag="scores")
                    nc.vector.reciprocal(scores, sc1)

                    # p_unnorm = exp(t), row-sum fused; w = p_unnorm*(scores/rowsum)
                    p_un = pAs.tile([128, K], F32, tag="pun")
                    rs = pAs.tile([128, 1], F32, tag="rs")
                    nc.scalar.activation(p_un, t_sb, AF.Exp, accum_out=rs)
                    rrs = pAs.tile([128, 1], F32, tag="rrs")
                    nc.vector.reciprocal(rrs, rs)
                    s2 = pAs.tile([128, 1], F32, tag="s2")
                    nc.vector.tensor_tensor(s2, scores, rrs, ALU.mult)
                    w_sb = pAs.tile([128, K], F32R, tag="wsb")
                    nc.vector.tensor_scalar_mul(w_sb, p_un, s2)

                    # wv_sb = [ones | write_vals]
                    wv_sb = pAs.tile([128, DC + 2], F32R, tag="wvsb")
                    nc.vector.tensor_copy(out=wv_sb[:, 0:2], in_=ones_col2_f)
                    nc.vector.tensor_copy(out=wv_sb[:, 2:DC + 2], in_=ps_wv)
                    pending.append((i, w_sb, wv_sb))

                flush_updates()

                # ------- phase B: slot update, overwrite cache rows -------
                base_t = LAYER_IDX * K // 128  # n-tile 6
                for kc in range(2):
                    mass = pAs.tile([128, 1], F32, tag="mass")
                    nc.vector.tensor_copy(out=mass, in_=ps_ua[kc][:, 0:1])
                    m1 = pAs.tile([128, 1], F32, tag="m1")
                    nc.vector.tensor_scalar_add(m1, mass, EPS)
                    rm = pAs.tile([128, 1], F32, tag="rm")
                    nc.vector.reciprocal(rm, m1)
                    m2 = pAs.tile([128, 1], F32, tag="m2")
                    nc.vector.tensor_scalar_add(m2, mass, 1.0)
                    rg = pAs.tile([128, 1], F32, tag="rg")
                    nc.vector.reciprocal(rg, m2)
                    g_t = pAs.tile([128, 1], F32, tag="gt")
                    nc.vector.tensor_tensor(g_t, mass, rg, ALU.mult)
                    co = pAs.tile([128, 1], F32, tag="co")
                    nc.vector.tensor_scalar(co, g_t, -DECAY, DECAY, ALU.mult, ALU.add)
                    cn = pAs.tile([128, 1], F32, tag="cn")
                    nc.vector.tensor_tensor(cn, g_t, rm, ALU.mult)

                    told = pAs.tile([128, DC], F32, tag="told")
                    nc.vector.tensor_scalar_mul(told, cache_sb[:, base_t + kc, :], co)
                    nc.vector.scalar_tensor_tensor(
                        out=cache_sb[:, base_t + kc, 0:K],
                        in0=ps_ua[kc][:, 2:K + 2], scalar=cn, in1=told[:, 0:K],
                        op0=ALU.mult, op1=ALU.add,
                    )
                    nc.vector.scalar_tensor_tensor(
                        out=cache_sb[:, base_t + kc, K:DC],
                        in0=ps_ub[kc], scalar=cn, in1=told[:, K:DC],
                        op0=ALU.mult, op1=ALU.add,
                    )

            # ---------------- phases C-F ----------------------------------
            with (
                tc.tile_pool(name="woP", bufs=1) as woP,
                tc.tile_pool(name="aoP", bufs=1) as aoP,
            ):
                wo_sb = woP.tile([128, H, DC], BF16)
                nc.gpsimd.dma_start(out=wo_sb, in_=a["Wo"].rearrange("(c p) d -> p c d", p=128))
                aoT = aoP.tile([128, H, N], BF16)

                with (
                    tc.tile_pool(name="c2tP", bufs=1) as c2tP,
                    tc.tile_pool(name="wqkvP", bufs=1) as wqkvP,
                    tc.tile_pool(name="qkvP", bufs=1) as qkvP,
                ):
                    # ------- phase C: cache2 -> cache2T (bf16) -----------
                    c2t = c2tP.tile([128, 4, N], BF16)
                    with tc.tile_pool(name="psC", bufs=2, space="PSUM") as psC:
                        for j in range(4):
                            for tg in range(4):
                                ps = psC.tile([128, 512], F32, tag="ctr")
                                for tt in range(4):
                                    t = tg * 4 + tt
                                    nc.tensor.transpose(
                                        ps[:, tt * 128:(tt + 1) * 128],
                                        cache_sb[:, t, j * 128:(j + 1) * 128],
                                        ident,
                                    )
                                nc.scalar.copy(
                                    out=c2t[:, j, tg * 512:(tg + 1) * 512], in_=ps)

                    # ------- phase D: q/k/v projections (bf16) -----------
                    wq_sb = wqkvP.tile([128, 4, DC], BF16)
                    nc.gpsimd.dma_start(out=wq_sb, in_=a["Wq"].rearrange("(c p) d -> p c d", p=128))
                    wk_sb = wqkvP.tile([128, 4, DC], BF16)
                    nc.gpsimd.dma_start(out=wk_sb, in_=a["Wk"].rearrange("(c p) d -> p c d", p=128))
                    wv_w = wqkvP.tile([128, 4, DC], BF16)
                    nc.gpsimd.dma_start(out=wv_w, in_=a["Wv"].rearrange("(c p) d -> p c d", p=128))

                    qT = qkvP.tile([128, H, N], BF16)
                    kT = qkvP.tile([128, H, N], BF16)
                    v_sb = qkvP.tile([128, NT, DC], BF16)

                    with tc.tile_pool(name="psD", bufs=3, space="PSUM") as psD:
                        for dst, w_t, b_t in ((qT, wq_sb, bqr), (kT, wk_sb, bkr)):
                            for h in range(H):
                                for c in range(4):
                                    ps = psD.tile([128, 512], F32, tag="qk")
                                    for j in range(4):
                                        nc.tensor.matmul(
                                            ps, w_t[:, j, h * 128:(h + 1) * 128],
                                            c2t[:, j, c * 512:(c + 1) * 512],
                                            start=(j == 0), stop=False,
                                        )
                                    nc.tensor.matmul(
                                        ps, b_t[:, h * 128:(h + 1) * 128],
                                        ones_row_bf[:, 0:512], start=False, stop=True,
                                    )
                                    nc.scalar.copy(
                                        out=dst[:, h, c * 512:(c + 1) * 512], in_=ps)
                        for m in range(NT):
                            ps = psD.tile([128, DC], F32, tag="v")
                            for j in range(4):
                                nc.tensor.matmul(
                                    ps, c2t[:, j, m * 128:(m + 1) * 128], wv_w[:, j, :],
                                    start=(j == 0), stop=False,
                                )
                            nc.tensor.matmul(ps, ones_row_bf[:, 0:128], bvr,
                                             start=False, stop=True)
                            nc.scalar.copy(out=v_sb[:, m, :], in_=ps)

                    # ------- phase E: attention (transposed layout) ------
                    with (
                        tc.tile_pool(name="pE", bufs=3) as pE,
                        tc.tile_pool(name="pEs", bufs=2) as pEs,
                        tc.tile_pool(name="psAtt", bufs=2, space="PSUM") as psAtt,
                        tc.tile_pool(name="psAo", bufs=1, space="PSUM") as psAo,
                        tc.tile_pool(name="psDen", bufs=2, space="PSUM") as psDen,
                    ):
                        ps_aoA = psAo.tile([128, 512], F32, tag="aoA")
                        ps_aoB = psAo.tile([128, 512], F32, tag="aoB")
                        for c in range(NCH):
                            ps_den = psDen.tile([128, CL], F32, tag="den")
                            for m in range(NT):
                                ps_a = psAtt.tile([128, H * CL], F32, tag="att")
                                for h in range(H):
                                    nc.tensor.matmul(
                                        ps_a[:, h * CL:(h + 1) * CL],
                                        kT[:, h, m * 128:(m + 1) * 128],
                                        qT[:, h, c * CL:(c + 1) * CL],
                                        start=True, stop=True,
                                    )
                                pT = pE.tile([128, H * CL], BF16, tag="pT")
                                nc.scalar.activation(pT, ps_a, AF.Exp, scale=ATT_SCALE)
                                for h in range(H):
                                    ps_ao = ps_aoA if h < 2 else ps_aoB
                                    # one accumulation group per PSUM bank: the
                                    # bank-wide zero region from the h-even start
                                    # makes the h-odd m==0 write an overwrite.
                                    nc.tensor.matmul(
                                        ps_ao[:, (h % 2) * CL:(h % 2 + 1) * CL],
                                        v_sb[:, m, h * 128:(h + 1) * 128],
                                        pT[:, h * CL:(h + 1) * CL],
                                        start=(m == 0 and h % 2 == 0),
                                        stop=(m == NT - 1 and h % 2 == 1),
                                    )
                                for h in range(H):
                                    nc.tensor.matmul(
                                        ps_den[32 * h:32 * h + 1, :],
                                        ones_col_bf,
                                        pT[:, h * CL:(h + 1) * CL],
                                        start=(m == 0), stop=(m == NT - 1),
                                        tile_position=(0, 32 * h),
                                    )
                            # free the ao banks fast: unnormalized copy to SBUF
                            aoU = pEs.tile([128, H * CL], F32, tag="aoU")
                            nc.vector.tensor_copy(out=aoU[:, 0:2 * CL], in_=ps_aoA)
                            nc.vector.tensor_copy(out=aoU[:, 2 * CL:4 * CL], in_=ps_aoB)
                            # denominators -> 1/den on DVE (keeps the Exp table resident)
                            den_sb = pEs.tile([128, CL], F32, tag="densb")
                            nc.vector.memset(den_sb, 1.0)
                            for h in range(H):
                                nc.vector.tensor_copy(
                                    out=den_sb[32 * h:32 * h + 1, :],
                                    in_=ps_den[32 * h:32 * h + 1, :],
                                )
                            rden_sb = pEs.tile([128, CL], F32, tag="rdensb")
                            nc.vector.reciprocal(rden_sb, den_sb)
                            rden = pEs.tile([1, H * CL], F32, tag="rden")
                            for h in range(H):
                                nc.sync.dma_start(
                                    out=rden[:, h * CL:(h + 1) * CL],
                                    in_=rden_sb[32 * h:32 * h + 1, :],
                                )
                            bc_sb = pEs.tile([128, H * CL], F32, tag="bcsb")
                            nc.gpsimd.partition_broadcast(bc_sb, rden)
                            for h in range(H):
                                nc.vector.scalar_tensor_tensor(
                                    out=aoT[:, h, c * CL:(c + 1) * CL],
                                    in0=aoU[:, h * CL:(h + 1) * CL],
                                    scalar=1.0, in1=bc_sb[:, h * CL:(h + 1) * CL],
                                    op0=ALU.mult, op1=ALU.mult,
                                )

                # ------- phase F: o-proj + residual + layernorm ----------
                with (
                    tc.tile_pool(name="pF", bufs=2) as pF,
                    tc.tile_pool(name="pFbig", bufs=1) as pFbig,
                    tc.tile_pool(name="psF", bufs=2, space="PSUM") as psF,
                ):
                    r_all = pFbig.tile([128, NT, DC], F32)
                    mean_all = pFbig.tile([128, NT], F32)
                    ssq_all = pFbig.tile([128, NT], F32)
                    for t in range(NT):
                        ps_o = psF.tile([128, DC], F32, tag="o")
                        for h in range(H):
                            nc.tensor.matmul(
                                ps_o, aoT[:, h, t * 128:(t + 1) * 128], wo_sb[:, h, :],
                                start=(h == 0), stop=False,
                            )
                        nc.tensor.matmul(ps_o, ones_row_bf[:, 0:128], bor,
                                         start=False, stop=True)
                        rsum = pF.tile([128, 1], F32, tag="rsum")
                        nc.vector.scalar_tensor_tensor(
                            out=r_all[:, t, :], in0=ps_o, scalar=1.0,
                            in1=cache_sb[:, t, :],
                            op0=ALU.mult, op1=ALU.add, accum_out=rsum,
                        )
                        nc.vector.tensor_scalar_mul(
                            mean_all[:, t:t + 1], rsum, 1.0 / DC)
                        scratch = pF.tile([128, DC], F32, tag="scratch")
                        nc.vector.scalar_tensor_tensor(
                            out=scratch, in0=r_all[:, t, :],
                            scalar=mean_all[:, t:t + 1], in1=r_all[:, t, :],
                            op0=ALU.subtract, op1=ALU.mult,
                            accum_out=ssq_all[:, t:t + 1],
                        )
                    lnv_all = pFbig.tile([128, NT], F32)
                    nc.scalar.activation(lnv_all, ssq_all, AF.Ln, scale=1.0 / DC,
                                         bias=eps5_t)
                    rstd_all = pFbig.tile([128, NT], F32)
                    nc.scalar.activation(rstd_all, lnv_all, AF.Exp, scale=-0.5)
                    for t in range(NT):
                        t1 = pF.tile([128, DC], F32, tag="t1")
                        nc.vector.tensor_scalar(
                            t1, r_all[:, t, :], mean_all[:, t:t + 1],
                            rstd_all[:, t:t + 1], ALU.subtract, ALU.mult)
                        t2 = pF.tile([128, DC], F32, tag="t2")
                        nc.vector.scalar_tensor_tensor(
                            out=t2, in0=t1, scalar=1.0, in1=lng_bc,
                            op0=ALU.mult, op1=ALU.mult,
                        )
                        o_sb = pF.tile([128, DC], F32, tag="osb")
                        nc.vector.scalar_tensor_tensor(
                            out=o_sb, in0=t2, scalar=1.0, in1=lnb_bc,
                            op0=ALU.mult, op1=ALU.add,
                        )
                        nc.sync.dma_start(out=out3[:, t, :], in_=o_sb)

    nc.compile()
    return nc


_NC_CACHE = {}


def _get_nc():
    if "nc" not in _NC_CACHE:
        _NC_CACHE["nc"] = _build()
    return _NC_CACHE["nc"]


def _in_maps(inputs):
    per_batch = {"y", "cache", "gumbel_u"}
    maps = []
    for b in range(B):
        m = {}
        for name in _INPUT_SPECS:
            arr = np.ascontiguousarray(np.asarray(inputs[name], dtype=np.float32))
            m[name] = arr[b] if name in per_batch else arr
        maps.append(m)
    return maps


def _execute(inputs, trace=False):
    nc = _get_nc()
    res = run_bass_kernel_spmd(nc, _in_maps(inputs), list(range(B)), trace=trace)
    out = np.stack([res.results[b]["out"] for b in range(B)]).astype(np.float32)
    return out, res


def kernel(**inputs) -> np.ndarray:
    out, _ = _execute(inputs)
    return out
